# revision 11
# baseline (speedup 1.0000x reference)
"""Trainium2 Bass kernel for nn_CharAttention (causal single-head attention, T=4096, D=1024).

Strategy (8 NeuronCores, SPMD):
  - Queries sharded across cores with a balanced causal interleave: core c owns
    global 128-row q-blocks {c, 15-c, 16+c, 31-c} ("slots" 0..3), so every core
    does the same causal work (structurally identical static program).
  - k/v computed shard-wise (core c projects rows [512c, 512c+512)) then
    AllGather'd in fp16/bf16.
  - Slot s is padded to (s+1)*1024 key-columns; the data-dependent causal
    boundary is applied with an iota>pos additive -1e9 mask on the last quarter
    of each slot (the diagonal always lands there for every core).
  - Softmax without a running max: m_hat = rowmax(first 1024 cols) + 50.
    exp(s - m_hat) stays within bf16/f32 range (margin analysis: overflow needs
    a later-quarter score 138 above the quarter-0 max; underfow drops only
    weights < 1e-16 of the total), so quarter contributions accumulate with
    plain adds and one final 1/l normalization.
  - dtypes: q/k/scores chain in fp16 (PE full rate, 8x finer mantissa than
    bf16 -- scores have std ~32 and softmax is argmax-sensitive); p/v/attn/proj
    in bf16 (needs fp32-wide exponent range for the shifted exp).
"""

import numpy as np
import ml_dtypes

T = 4096
D = 1024
N_CORES = 8
NBLK = T // 128  # 32 global q-blocks
DELTA = 50.0
NEG_BIG = -1e9

# slot assignment: core c -> global blocks [c, 15-c, 16+c, 31-c]
def core_blocks(c):
    return [c, 15 - c, 16 + c, 31 - c]

PADQ = [1, 2, 3, 4]  # quarters (1024 cols) computed per slot

FILL_AGK = 110  # filler matmuls (~213ns each) bridging the AG(k) wait
FILL_AGV = 60  # filler matmuls bridging the AG(v) wait

_COMPILED = None
LAST_EXEC_NS = None


def _build():
    import concourse.bass as bass
    import concourse.mybir as mybir
    from concourse import bacc
    from concourse.tile import TileContext
    from concourse.masks import make_identity

    f16, bf16, f32 = mybir.dt.float16, mybir.dt.bfloat16, mybir.dt.float32
    AT = mybir.ActivationFunctionType
    OP = mybir.AluOpType
    AX = mybir.AxisListType

    nc = bacc.Bacc("TRN2", target_bir_lowering=False, debug=False, num_devices=N_CORES)

    # --- I/O ---
    xqT_d = nc.dram_tensor("xqT", [D, 512], f16, kind="ExternalInput")
    xkvT_d = nc.dram_tensor("xkvT", [D, 512], f16, kind="ExternalInput")
    xqres_d = nc.dram_tensor("xqres", [512, D], f32, kind="ExternalInput")
    pos_d = nc.dram_tensor("pos", [128, 4], f32, kind="ExternalInput")
    wqT_d = nc.dram_tensor("wqT", [D, D], f16, kind="ExternalInput")
    wkT_d = nc.dram_tensor("wkT", [D, D], f16, kind="ExternalInput")
    wvT_d = nc.dram_tensor("wvT", [D, D], f16, kind="ExternalInput")
    wpT_d = nc.dram_tensor("wpT", [D, D], bf16, kind="ExternalInput")
    out_d = nc.dram_tensor("out", [512, D], f32, kind="ExternalOutput")
    attn_d = nc.dram_tensor("attn", [512, D], f32, kind="ExternalOutput")

    # internal DRAM for the collective
    ktloc_d = nc.dram_tensor("ktloc", [D, 512], f16)
    vloc_d = nc.dram_tensor("vloc", [512, D], bf16)
    ktag_d = nc.dram_tensor("ktag", [N_CORES, D, 512], f16, addr_space="Shared")
    vag_d = nc.dram_tensor("vag", [T, D], bf16, addr_space="Shared")

    groups = [list(range(N_CORES))]

    with TileContext(nc) as tc:
        with (
            tc.tile_pool(name="persist", bufs=1) as pp,
            tc.tile_pool(name="psum", bufs=1, space="PSUM") as psp,
        ):
            # persistent tiles
            warm_sb = pp.tile([128, 512], f16, tag="warm_sb", name="warm_sb")
            nc.vector.memset(warm_sb[:], 0.0)
            # HAM warmup: real matmuls (transposes don't count as PE-busy for
            # HAM) with no DMA deps, so the PE reaches K=8/8 before k_proj and
            # stays busy during the input-DMA lead-in.
            for w in range(6):
                wps = psp.tile([128, 512], f32, tag="pp", name="pp", bufs=3)
                nc.tensor.matmul(wps[:], warm_sb[:, 0:128], warm_sb[:], start=True, stop=True)

            qt = [pp.tile([128, 512], f16, tag=f"qt{e}", name=f"qt{e}") for e in range(8)]
            ident = pp.tile([128, 128], bf16, tag="ident", name="ident")
            make_identity(nc, ident[:])
            iota_i = pp.tile([128, 512], mybir.dt.int32, tag="iota_i", name="iota_i")
            nc.gpsimd.iota(iota_i[:], pattern=[[1, 512]], base=0, channel_multiplier=0)
            iota_f = pp.tile([128, 512], f32, tag="iota_f", name="iota_f")
            nc.vector.tensor_copy(iota_f[:], iota_i[:])
            pos_sb = pp.tile([128, 4], f32, tag="pos_sb", name="pos_sb")
            nc.sync.dma_start(out=pos_sb[:], in_=pos_d[:])
            A_sb = [pp.tile([128, D], f32, tag=f"A{s}", name=f"A{s}") for s in range(4)]
            lpart = [pp.tile([128, 4], f32, tag=f"lp{s}", name=f"lp{s}") for s in range(4)]
            negm = [pp.tile([128, 1], f32, tag=f"nm{s}", name=f"nm{s}") for s in range(4)]

            # ---------------- Phase A: projections + allgather ----------------
            with tc.tile_pool(name="pha", bufs=1) as pa:
                # merged tiles; one dma_start each (split across all 16 SDMA
                # engines) to avoid the ~0.6us-per-dispatch serial sequencer cost
                xkv = pa.tile([128, 4096], f16, tag="xkv", name="xkv")
                wkb = pa.tile([128, 8192], f16, tag="wkb", name="wkb")
                nc.sync.dma_start(out=xkv[:].rearrange("p (d t) -> p d t", d=8),
                                  in_=xkvT_d[:].rearrange("(d p) t -> p d t", p=128))
                nc.sync.dma_start(out=wkb[:].rearrange("p (d e) -> p d e", d=8),
                                  in_=wkT_d[:].rearrange("(d p) e -> p d e", p=128))
                wvb = pa.tile([128, 8192], f16, tag="wvb", name="wvb")
                xqb = pa.tile([128, 4096], f16, tag="xqb", name="xqb")
                wqb = pa.tile([128, 8192], f16, tag="wqb", name="wqb")

                ktbig = pa.tile([128, 4096], f16, tag="ktbig", name="ktbig")
                with nc.named_scope("k_proj"):
                    # kT_local[e,t] = sum_d WkT[d,e] * xkvT[d,t]
                    for e in range(8):
                        ps = psp.tile([128, 512], f32, tag="pp", name="pp", bufs=3)
                        for d in range(8):
                            nc.tensor.matmul(
                                ps[:], wkb[:, 1024 * d + 128 * e:1024 * d + 128 * (e + 1)],
                                xkv[:, 512 * d:512 * (d + 1)],
                                start=(d == 0), stop=(d == 7),
                            )
                        nc.scalar.copy(ktbig[:, 512 * e:512 * (e + 1)], ps[:])
                        if e == 0:
                            # dispatch the remaining input loads only now, so the
                            # k-path inputs get exclusive HBM bandwidth first
                            nc.scalar.dma_start(out=wvb[:].rearrange("p (d e) -> p d e", d=8),
                                                in_=wvT_d[:].rearrange("(d p) e -> p d e", p=128))
                            nc.scalar.dma_start(out=xqb[:].rearrange("p (d t) -> p d t", d=8),
                                                in_=xqT_d[:].rearrange("(d p) t -> p d t", p=128))
                            nc.scalar.dma_start(out=wqb[:].rearrange("p (d e) -> p d e", d=8),
                                                in_=wqT_d[:].rearrange("(d p) e -> p d e", p=128))
                        if e == 3:
                            nc.sync.dma_start(
                                out=ktloc_d[:].rearrange("(e p) t -> p e t", p=128)[:, 0:4, :],
                                in_=ktbig[:, 0:2048].rearrange("p (e t) -> p e t", e=4))
                    nc.sync.dma_start(
                        out=ktloc_d[:].rearrange("(e p) t -> p e t", p=128)[:, 4:8, :],
                        in_=ktbig[:, 2048:4096].rearrange("p (e t) -> p e t", e=4))

                nc.gpsimd.collective_compute(
                    "AllGather", mybir.AluOpType.bypass, replica_groups=groups,
                    ins=[ktloc_d[:]], outs=[ktag_d[:]],
                )

                vbig = pa.tile([128, 4096], bf16, tag="vbig", name="vbig")
                with nc.named_scope("v_proj"):
                    # v_local[t,e] = sum_d xkvT[d,t] * WvT[d,e]
                    for t in range(4):
                        for h in range(2):
                            ps = psp.tile([128, 512], f32, tag="pp", name="pp", bufs=3)
                            for d in range(8):
                                nc.tensor.matmul(
                                    ps[:], xkv[:, 512 * d + 128 * t:512 * d + 128 * (t + 1)],
                                    wvb[:, 1024 * d + 512 * h:1024 * d + 512 * (h + 1)],
                                    start=(d == 0), stop=(d == 7),
                                )
                            nc.scalar.copy(vbig[:, 1024 * t + 512 * h:1024 * t + 512 * (h + 1)], ps[:])
                        if t == 1:
                            nc.sync.dma_start(
                                out=vloc_d[:].rearrange("(t p) e -> p t e", p=128)[:, 0:2, :],
                                in_=vbig[:, 0:2048].rearrange("p (t e) -> p t e", t=2))
                    nc.sync.dma_start(
                        out=vloc_d[:].rearrange("(t p) e -> p t e", p=128)[:, 2:4, :],
                        in_=vbig[:, 2048:4096].rearrange("p (t e) -> p t e", t=2))

                nc.gpsimd.collective_compute(
                    "AllGather", mybir.AluOpType.bypass, replica_groups=groups,
                    ins=[vloc_d[:]], outs=[vag_d[:]],
                )

                with nc.named_scope("q_proj"):
                    for e in range(8):
                        ps = psp.tile([128, 512], f32, tag="pp", name="pp", bufs=3)
                        for d in range(8):
                            nc.tensor.matmul(
                                ps[:], wqb[:, 1024 * d + 128 * e:1024 * d + 128 * (e + 1)],
                                xqb[:, 512 * d:512 * (d + 1)],
                                start=(d == 0), stop=(d == 7),
                            )
                        nc.scalar.copy(qt[e][:], ps[:])

                # filler matmuls: keep the PE at K=8/8 while AG(k) finishes
                with nc.named_scope("fill_agk"):
                    for w in range(FILL_AGK):
                        wps = psp.tile([128, 512], f32, tag="pp", name="pp", bufs=3)
                        nc.tensor.matmul(wps[:], warm_sb[:, 0:128], warm_sb[:], start=True, stop=True)

            # ---------------- Phase B: attention over quarters ----------------
            with tc.tile_pool(name="phb", bufs=1) as pb:
                wproj = [pb.tile([128, D], bf16, tag=f"wp{d}", name=f"wp{d}") for d in range(8)]
                for d in range(8):
                    nc.sync.dma_start(out=wproj[d][:], in_=wpT_d[128 * d:128 * (d + 1), :])

                # ---- Pass 1: all scores + exp + P-transposes (overlaps v-AllGather) ----
                pt_tiles = {}
                for qtr in range(4):
                    with nc.named_scope(f"sc{qtr}"):
                        kq = [pb.tile([128, 1024], f16, tag=f"kq{e}", bufs=4, name=f"kq{e}") for e in range(8)]
                        for e in range(8):
                            for hh in range(2):
                                nc.sync.dma_start(
                                    out=kq[e][:, 512 * hh:512 * (hh + 1)],
                                    in_=ktag_d[2 * qtr + hh, 128 * e:128 * (e + 1), :],
                                )
                        for s in range(4):
                            if qtr >= PADQ[s]:
                                continue
                            last_q = (qtr == PADQ[s] - 1)
                            ssb = pb.tile([128, 1024], f32, tag="ssb", bufs=2, name="ssb")
                            for pn in range(2):
                                ps = psp.tile([128, 512], f32, tag="pp", name="pp", bufs=3)
                                for e in range(8):
                                    nc.tensor.matmul(
                                        ps[:], qt[e][:, 128 * s:128 * (s + 1)],
                                        kq[e][:, 512 * pn:512 * (pn + 1)],
                                        start=(e == 0), stop=(e == 7),
                                    )
                                if last_q:
                                    shift = pb.tile([128, 1], f32, tag="shift", bufs=2, name="shift")
                                    nc.vector.tensor_scalar_add(
                                        shift[:], pos_sb[:, s:s + 1],
                                        float(-(qtr * 1024 + pn * 512)),
                                    )
                                    madd = pb.tile([128, 512], f32, tag="madd", bufs=1, name="madd")
                                    nc.vector.tensor_scalar(
                                        out=madd[:], in0=iota_f[:], scalar1=shift[:, 0:1],
                                        scalar2=NEG_BIG, op0=OP.is_gt, op1=OP.mult,
                                    )
                                    nc.vector.tensor_tensor(
                                        out=ssb[:, 512 * pn:512 * (pn + 1)], in0=ps[:],
                                        in1=madd[:], op=OP.add,
                                    )
                                else:
                                    nc.scalar.copy(ssb[:, 512 * pn:512 * (pn + 1)], ps[:])
                            if qtr == 0:
                                nc.vector.reduce_max(
                                    negm[s][:], ssb[:], axis=AX.X, negate=True)
                                nc.vector.tensor_scalar_add(negm[s][:], negm[s][:], -DELTA)
                            psb = pb.tile([128, 1024], bf16, tag="psb", bufs=3, name="psb")
                            nc.scalar.activation(
                                psb[:], ssb[:], AT.Exp, bias=negm[s][:, 0:1], scale=1.0,
                                accum_out=lpart[s][:, qtr:qtr + 1],
                            )
                            ps_tr = psp.tile([128, 1024], bf16, tag="ps_trrow", bufs=1, name="ps_trrow")
                            for j in range(8):
                                nc.tensor.transpose(
                                    ps_tr[:, 128 * j:128 * (j + 1)],
                                    psb[:, 128 * j:128 * (j + 1)], ident[:])
                            pt_sb = pb.tile([128, 1024], bf16, tag="pt_sb", bufs=9, name="pt_sb")
                            nc.scalar.copy(pt_sb[:], ps_tr[:])
                            pt_tiles[(s, qtr)] = pt_sb

                # filler matmuls: keep the PE warm while AG(v) finishes
                with nc.named_scope("fill_agv"):
                    for w in range(FILL_AGV):
                        wps = psp.tile([128, 512], f32, tag="pp", name="pp", bufs=3)
                        nc.tensor.matmul(wps[:], warm_sb[:, 0:128], warm_sb[:], start=True, stop=True)

                # ---- Pass 2: att @ v per quarter, then per-slot finalize ----
                for qtr in range(4):
                    with nc.named_scope(f"av{qtr}"):
                        vq = [pb.tile([128, 1024], bf16, tag=f"vq{j}", bufs=2, name=f"vq{j}") for j in range(8)]
                        for j in range(8):
                            nc.sync.dma_start(
                                out=vq[j][:],
                                in_=vag_d[1024 * qtr + 128 * j:1024 * qtr + 128 * (j + 1), :],
                            )
                        for s in range(4):
                            if qtr >= PADQ[s]:
                                continue
                            last_q = (qtr == PADQ[s] - 1)
                            pt_sb = pt_tiles[(s, qtr)]
                            ps_av = psp.tile([128, 1024], f32, tag="ps_av", name="ps_av", bufs=2)
                            for h in range(2):
                                for j in range(8):
                                    nc.tensor.matmul(
                                        ps_av[:, 512 * h:512 * (h + 1)],
                                        pt_sb[:, 128 * j:128 * (j + 1)],
                                        vq[j][:, 512 * h:512 * (h + 1)],
                                        start=(j == 0), stop=(j == 7),
                                    )
                            if qtr == 0:
                                nc.vector.tensor_copy(A_sb[s][:], ps_av[:])
                            else:
                                nc.vector.tensor_tensor(
                                    out=A_sb[s][:], in0=A_sb[s][:], in1=ps_av[:], op=OP.add)

                            # ---------- finalize slot after its last quarter ----------
                            if last_q:
                                with nc.named_scope(f"fin{s}"):
                                    lsum = pb.tile([128, 1], f32, tag="lsum", bufs=2, name="lsum")
                                    if PADQ[s] > 1:
                                        nc.vector.reduce_sum(
                                            lsum[:], lpart[s][:, 0:PADQ[s]], axis=AX.X)
                                    else:
                                        nc.vector.tensor_copy(lsum[:], lpart[s][:, 0:1])
                                    rl = pb.tile([128, 1], f32, tag="rl", bufs=2, name="rl")
                                    nc.vector.reciprocal(rl[:], lsum[:])
                                    attn_f = pb.tile([128, D], f32, tag="attn_f", bufs=2, name="attn_f")
                                    nc.scalar.activation(
                                        attn_f[:], A_sb[s][:], AT.Copy, bias=0.0,
                                        scale=rl[:, 0:1])
                                    nc.sync.dma_start(
                                        out=attn_d[128 * s:128 * (s + 1), :], in_=attn_f[:])
                                    attn_b = pb.tile([128, D], bf16, tag="attn_b", bufs=2, name="attn_b")
                                    nc.vector.tensor_copy(attn_b[:], attn_f[:])
                                    ps_t2 = psp.tile([128, 1024], bf16, tag="ps_trrow", bufs=1, name="ps_trrow")
                                    for ec in range(8):
                                        nc.tensor.transpose(
                                            ps_t2[:, 128 * ec:128 * (ec + 1)],
                                            attn_b[:, 128 * ec:128 * (ec + 1)],
                                            ident[:])
                                    at_row = pb.tile([128, 1024], bf16, tag="at_sb", bufs=2, name="at_sb")
                                    nc.scalar.copy(at_row[:], ps_t2[:])
                                    xqr = pb.tile([128, D], f32, tag="xqr", bufs=2, name="xqr")
                                    nc.sync.dma_start(
                                        out=xqr[:], in_=xqres_d[128 * s:128 * (s + 1), :])
                                    out_sb = pb.tile([128, D], f32, tag="out_sb", bufs=2, name="out_sb")
                                    for h in range(2):
                                        ps_o = psp.tile([128, 512], f32, tag="pp", name="pp", bufs=3)
                                        for ec in range(8):
                                            nc.tensor.matmul(
                                                ps_o[:], at_row[:, 128 * ec:128 * (ec + 1)],
                                                wproj[ec][:, 512 * h:512 * (h + 1)],
                                                start=(ec == 0), stop=(ec == 7),
                                            )
                                        nc.vector.tensor_tensor(
                                            out=out_sb[:, 512 * h:512 * (h + 1)], in0=ps_o[:],
                                            in1=xqr[:, 512 * h:512 * (h + 1)], op=OP.add)
                                    nc.sync.dma_start(
                                        out=out_d[128 * s:128 * (s + 1), :], in_=out_sb[:])

    nc.compile()
    return nc


def _get_compiled():
    global _COMPILED
    if _COMPILED is None:
        _COMPILED = _build()
    return _COMPILED


def kernel(x, attention_mask, Wq, Wkv, Wproj, _trace=False):
    global LAST_EXEC_NS
    from concourse.bass_utils import run_bass_kernel_spmd

    x = np.asarray(x)
    attention_mask = np.asarray(attention_mask)
    Wq, Wkv, Wproj = np.asarray(Wq), np.asarray(Wkv), np.asarray(Wproj)
    assert x.shape == (T, D) and attention_mask.shape == (T,)
    assert np.array_equal(attention_mask, np.arange(T, dtype=attention_mask.dtype)), \
        "kernel assumes attention_mask == arange(T)"

    x16 = x.astype(np.float16)
    wqT = np.ascontiguousarray(Wq.T).astype(np.float16)
    wkT = np.ascontiguousarray(Wkv[:D].T).astype(np.float16)
    wvT = np.ascontiguousarray(Wkv[D:].T).astype(np.float16)
    wpT = np.ascontiguousarray(Wproj.T).astype(ml_dtypes.bfloat16)

    in_maps = []
    core_rows = []
    for c in range(N_CORES):
        blocks = core_blocks(c)
        rows = np.concatenate([np.arange(128 * b, 128 * (b + 1)) for b in blocks])
        core_rows.append(rows)
        xq = x[rows]                      # [512, D] f32
        xqT = np.ascontiguousarray(x16[rows].T)    # [D, 512] f16
        xkvT = np.ascontiguousarray(x16[512 * c:512 * (c + 1)].T)
        pos = np.empty((128, 4), np.float32)
        for s, b in enumerate(blocks):
            pos[:, s] = 128 * b + np.arange(128)
        in_maps.append({
            "xqT": xqT, "xkvT": xkvT,
            "xqres": np.ascontiguousarray(xq.astype(np.float32)),
            "pos": pos,
            "wqT": wqT, "wkT": wkT, "wvT": wvT, "wpT": wpT,
        })

    nc = _get_compiled()
    res = run_bass_kernel_spmd(nc, in_maps, list(range(N_CORES)), trace=_trace)
    LAST_EXEC_NS = res.exec_time_ns
    globals()["LAST_RES"] = res

    out_full = np.empty((T, D), np.float32)
    x_new = x.astype(np.float32).copy()
    for c in range(N_CORES):
        r = res.results[c]
        out_full[core_rows[c]] = r["out"]
        x_new[core_rows[c]] += r["attn"]
    return out_full, x_new



# revision 13
# speedup vs baseline: 1.0574x; 1.0574x over previous
"""Trainium2 Bass kernel for nn_CharAttention (causal single-head attention, T=4096, D=1024).

Strategy (8 NeuronCores, SPMD):
  - Queries sharded across cores with a balanced causal interleave: core c owns
    global 128-row q-blocks {c, 15-c, 16+c, 31-c} ("slots" 0..3), so every core
    does the same causal work (structurally identical static program).
  - k/v computed shard-wise (core c projects rows [512c, 512c+512)) then
    AllGather'd in fp16/bf16.
  - Slot s is padded to (s+1)*1024 key-columns; the data-dependent causal
    boundary is applied with an iota>pos additive -1e9 mask on the last quarter
    of each slot (the diagonal always lands there for every core).
  - Softmax without a running max: m_hat = rowmax(first 1024 cols) + 50.
    exp(s - m_hat) stays within bf16/f32 range (margin analysis: overflow needs
    a later-quarter score 138 above the quarter-0 max; underfow drops only
    weights < 1e-16 of the total), so quarter contributions accumulate with
    plain adds and one final 1/l normalization.
  - dtypes: q/k/scores chain in fp16 (PE full rate, 8x finer mantissa than
    bf16 -- scores have std ~32 and softmax is argmax-sensitive); p/v/attn/proj
    in bf16 (needs fp32-wide exponent range for the shifted exp).
"""

import numpy as np
import ml_dtypes

T = 4096
D = 1024
N_CORES = 8
NBLK = T // 128  # 32 global q-blocks
DELTA = 50.0
NEG_BIG = -1e9

# slot assignment: core c -> global blocks [c, 15-c, 16+c, 31-c]
def core_blocks(c):
    return [c, 15 - c, 16 + c, 31 - c]

PADQ = [1, 2, 3, 4]  # quarters (1024 cols) computed per slot

FILL_AGK = 110  # filler matmuls (~213ns each) bridging the AG(k) wait
FILL_AGV = 60  # filler matmuls bridging the AG(v) wait

_COMPILED = None
LAST_EXEC_NS = None


def _build():
    import concourse.bass as bass
    import concourse.mybir as mybir
    from concourse import bacc
    from concourse.tile import TileContext
    from concourse.masks import make_identity

    f16, bf16, f32 = mybir.dt.float16, mybir.dt.bfloat16, mybir.dt.float32
    AT = mybir.ActivationFunctionType
    OP = mybir.AluOpType
    AX = mybir.AxisListType

    nc = bacc.Bacc("TRN2", target_bir_lowering=False, debug=False, num_devices=N_CORES)

    # --- I/O ---
    xqT_d = nc.dram_tensor("xqT", [D, 512], f16, kind="ExternalInput")
    xkvT_d = nc.dram_tensor("xkvT", [D, 512], f16, kind="ExternalInput")
    xqres_d = nc.dram_tensor("xqres", [512, D], f32, kind="ExternalInput")
    pos_d = nc.dram_tensor("pos", [128, 4], f32, kind="ExternalInput")
    wqT_d = nc.dram_tensor("wqT", [D, D], f16, kind="ExternalInput")
    wkT_d = nc.dram_tensor("wkT", [D, D], f16, kind="ExternalInput")
    wvT_d = nc.dram_tensor("wvT", [D, D], f16, kind="ExternalInput")
    wpT_d = nc.dram_tensor("wpT", [D, D], bf16, kind="ExternalInput")
    out_d = nc.dram_tensor("out", [512, D], f32, kind="ExternalOutput")
    attn_d = nc.dram_tensor("attn", [512, D], f32, kind="ExternalOutput")

    # internal DRAM for the collective
    ktloc_d = nc.dram_tensor("ktloc", [D, 512], f16)
    vloc_d = nc.dram_tensor("vloc", [512, D], bf16)
    ktag_d = nc.dram_tensor("ktag", [N_CORES, D, 512], f16, addr_space="Shared")
    vag_d = nc.dram_tensor("vag", [T, D], bf16, addr_space="Shared")
    dumloc_d = nc.dram_tensor("dumloc", [128, 4], f32)
    dumag_d = nc.dram_tensor("dumag", [N_CORES, 128, 4], f32, addr_space="Shared")

    groups = [list(range(N_CORES))]

    with TileContext(nc) as tc:
        with (
            tc.tile_pool(name="persist", bufs=1) as pp,
            tc.tile_pool(name="psum", bufs=1, space="PSUM") as psp,
        ):
            # persistent tiles
            warm_sb = pp.tile([128, 512], f16, tag="warm_sb", name="warm_sb")
            nc.vector.memset(warm_sb[:], 0.0)
            # HAM warmup: real matmuls (transposes don't count as PE-busy for
            # HAM) with no DMA deps, so the PE reaches K=8/8 before k_proj and
            # stays busy during the input-DMA lead-in.
            for w in range(6):
                wps = psp.tile([128, 512], f32, tag="pp", name="pp", bufs=3)
                nc.tensor.matmul(wps[:], warm_sb[:, 0:128], warm_sb[:], start=True, stop=True)

            qt = [pp.tile([128, 512], f16, tag=f"qt{e}", name=f"qt{e}") for e in range(8)]
            ident = pp.tile([128, 128], bf16, tag="ident", name="ident")
            make_identity(nc, ident[:])
            iota_i = pp.tile([128, 512], mybir.dt.int32, tag="iota_i", name="iota_i")
            nc.gpsimd.iota(iota_i[:], pattern=[[1, 512]], base=0, channel_multiplier=0)
            iota_f = pp.tile([128, 512], f32, tag="iota_f", name="iota_f")
            nc.vector.tensor_copy(iota_f[:], iota_i[:])
            pos_sb = pp.tile([128, 4], f32, tag="pos_sb", name="pos_sb")
            nc.sync.dma_start(out=pos_sb[:], in_=pos_d[:])
            A_sb = [pp.tile([128, D], f32, tag=f"A{s}", name=f"A{s}") for s in range(4)]
            lpart = [pp.tile([128, 4], f32, tag=f"lp{s}", name=f"lp{s}") for s in range(4)]
            negm = [pp.tile([128, 1], f32, tag=f"nm{s}", name=f"nm{s}") for s in range(4)]

            # dummy collective: absorbs the ~17us collective arming/sync cost
            # before the real AllGathers are triggered
            dum_sb = pp.tile([128, 4], f32, tag="dum_sb", name="dum_sb")
            nc.vector.memset(dum_sb[:], 0.0)
            nc.sync.dma_start(out=dumloc_d[:], in_=dum_sb[:])
            nc.gpsimd.collective_compute(
                "AllGather", mybir.AluOpType.bypass, replica_groups=groups,
                ins=[dumloc_d[:]], outs=[dumag_d[:]],
            )

            # ---------------- Phase A: projections + allgather ----------------
            with tc.tile_pool(name="pha", bufs=1) as pa:
                # per-chunk loads, dispatch spread across engine queues
                # (sync: xkvt, scalar: wk, vector: wv, gpsimd: xq/wq)
                xkvt = [pa.tile([128, 512], f16, tag=f"xkvt{d}", name=f"xkvt{d}") for d in range(8)]
                wk = [pa.tile([128, D], f16, tag=f"wk{d}", name=f"wk{d}") for d in range(8)]
                wv = [pa.tile([128, D], f16, tag=f"wv{d}", name=f"wv{d}") for d in range(8)]
                xqt = [pa.tile([128, 512], f16, tag=f"xqt{d}", name=f"xqt{d}") for d in range(8)]
                wq = [pa.tile([128, D], f16, tag=f"wq{d}", name=f"wq{d}") for d in range(8)]
                for d in range(8):
                    nc.sync.dma_start(out=xkvt[d][:], in_=xkvT_d[128 * d:128 * (d + 1), :])
                    nc.scalar.dma_start(out=wk[d][:], in_=wkT_d[128 * d:128 * (d + 1), :])
                for d in range(8):
                    nc.gpsimd.dma_start(out=wv[d][:], in_=wvT_d[128 * d:128 * (d + 1), :])
                for d in range(8):
                    nc.gpsimd.dma_start(out=xqt[d][:], in_=xqT_d[128 * d:128 * (d + 1), :])
                    nc.gpsimd.dma_start(out=wq[d][:], in_=wqT_d[128 * d:128 * (d + 1), :])

                with nc.named_scope("k_proj"):
                    # kT_local[e,t] = sum_d WkT[d,e] * xkvT[d,t]
                    for e in range(8):
                        ps = psp.tile([128, 512], f32, tag="pp", name="pp", bufs=3)
                        for d in range(8):
                            nc.tensor.matmul(
                                ps[:], wk[d][:, 128 * e:128 * (e + 1)], xkvt[d][:],
                                start=(d == 0), stop=(d == 7),
                            )
                        kt_sb = pa.tile([128, 512], f16, tag="kt_sb", name="kt_sb", bufs=3)
                        nc.scalar.copy(kt_sb[:], ps[:])
                        nc.sync.dma_start(out=ktloc_d[128 * e:128 * (e + 1), :], in_=kt_sb[:])

                nc.gpsimd.collective_compute(
                    "AllGather", mybir.AluOpType.bypass, replica_groups=groups,
                    ins=[ktloc_d[:]], outs=[ktag_d[:]],
                )

                with nc.named_scope("v_proj"):
                    # v_local[t,e] = sum_d xkvT[d,t] * WvT[d,e]
                    for t in range(4):
                        for h in range(2):
                            ps = psp.tile([128, 512], f32, tag="pp", name="pp", bufs=3)
                            for d in range(8):
                                nc.tensor.matmul(
                                    ps[:], xkvt[d][:, 128 * t:128 * (t + 1)],
                                    wv[d][:, 512 * h:512 * (h + 1)],
                                    start=(d == 0), stop=(d == 7),
                                )
                            v_sb = pa.tile([128, 512], bf16, tag="v_sb", name="v_sb", bufs=3)
                            nc.scalar.copy(v_sb[:], ps[:])
                            nc.sync.dma_start(
                                out=vloc_d[128 * t:128 * (t + 1), 512 * h:512 * (h + 1)],
                                in_=v_sb[:],
                            )

                nc.gpsimd.collective_compute(
                    "AllGather", mybir.AluOpType.bypass, replica_groups=groups,
                    ins=[vloc_d[:]], outs=[vag_d[:]],
                )

                with nc.named_scope("q_proj"):
                    for e in range(8):
                        ps = psp.tile([128, 512], f32, tag="pp", name="pp", bufs=3)
                        for d in range(8):
                            nc.tensor.matmul(
                                ps[:], wq[d][:, 128 * e:128 * (e + 1)], xqt[d][:],
                                start=(d == 0), stop=(d == 7),
                            )
                        nc.scalar.copy(qt[e][:], ps[:])

                # filler matmuls: keep the PE at K=8/8 while AG(k) finishes
                with nc.named_scope("fill_agk"):
                    for w in range(FILL_AGK):
                        wps = psp.tile([128, 512], f32, tag="pp", name="pp", bufs=3)
                        nc.tensor.matmul(wps[:], warm_sb[:, 0:128], warm_sb[:], start=True, stop=True)

            # ---------------- Phase B: attention over quarters ----------------
            with tc.tile_pool(name="phb", bufs=1) as pb:
                wproj = [pb.tile([128, D], bf16, tag=f"wp{d}", name=f"wp{d}") for d in range(8)]
                for d in range(8):
                    nc.sync.dma_start(out=wproj[d][:], in_=wpT_d[128 * d:128 * (d + 1), :])

                # ---- Pass 1: all scores + exp + P-transposes (overlaps v-AllGather) ----
                pt_tiles = {}
                for qtr in range(4):
                    with nc.named_scope(f"sc{qtr}"):
                        kq = [pb.tile([128, 1024], f16, tag=f"kq{e}", bufs=4, name=f"kq{e}") for e in range(8)]
                        for e in range(8):
                            for hh in range(2):
                                nc.sync.dma_start(
                                    out=kq[e][:, 512 * hh:512 * (hh + 1)],
                                    in_=ktag_d[2 * qtr + hh, 128 * e:128 * (e + 1), :],
                                )
                        for s in range(4):
                            if qtr >= PADQ[s]:
                                continue
                            last_q = (qtr == PADQ[s] - 1)
                            ssb = pb.tile([128, 1024], f32, tag="ssb", bufs=2, name="ssb")
                            for pn in range(2):
                                ps = psp.tile([128, 512], f32, tag="pp", name="pp", bufs=3)
                                for e in range(8):
                                    nc.tensor.matmul(
                                        ps[:], qt[e][:, 128 * s:128 * (s + 1)],
                                        kq[e][:, 512 * pn:512 * (pn + 1)],
                                        start=(e == 0), stop=(e == 7),
                                    )
                                if last_q:
                                    shift = pb.tile([128, 1], f32, tag="shift", bufs=2, name="shift")
                                    nc.vector.tensor_scalar_add(
                                        shift[:], pos_sb[:, s:s + 1],
                                        float(-(qtr * 1024 + pn * 512)),
                                    )
                                    madd = pb.tile([128, 512], f32, tag="madd", bufs=1, name="madd")
                                    nc.vector.tensor_scalar(
                                        out=madd[:], in0=iota_f[:], scalar1=shift[:, 0:1],
                                        scalar2=NEG_BIG, op0=OP.is_gt, op1=OP.mult,
                                    )
                                    nc.vector.tensor_tensor(
                                        out=ssb[:, 512 * pn:512 * (pn + 1)], in0=ps[:],
                                        in1=madd[:], op=OP.add,
                                    )
                                else:
                                    nc.scalar.copy(ssb[:, 512 * pn:512 * (pn + 1)], ps[:])
                            if qtr == 0:
                                nc.vector.reduce_max(
                                    negm[s][:], ssb[:], axis=AX.X, negate=True)
                                nc.vector.tensor_scalar_add(negm[s][:], negm[s][:], -DELTA)
                            psb = pb.tile([128, 1024], bf16, tag="psb", bufs=3, name="psb")
                            nc.scalar.activation(
                                psb[:], ssb[:], AT.Exp, bias=negm[s][:, 0:1], scale=1.0,
                                accum_out=lpart[s][:, qtr:qtr + 1],
                            )
                            ps_tr = psp.tile([128, 1024], bf16, tag="ps_trrow", bufs=1, name="ps_trrow")
                            for j in range(8):
                                nc.tensor.transpose(
                                    ps_tr[:, 128 * j:128 * (j + 1)],
                                    psb[:, 128 * j:128 * (j + 1)], ident[:])
                            pt_sb = pb.tile([128, 1024], bf16, tag="pt_sb", bufs=9, name="pt_sb")
                            nc.scalar.copy(pt_sb[:], ps_tr[:])
                            pt_tiles[(s, qtr)] = pt_sb

                # filler matmuls: keep the PE warm while AG(v) finishes
                with nc.named_scope("fill_agv"):
                    for w in range(FILL_AGV):
                        wps = psp.tile([128, 512], f32, tag="pp", name="pp", bufs=3)
                        nc.tensor.matmul(wps[:], warm_sb[:, 0:128], warm_sb[:], start=True, stop=True)

                # ---- Pass 2: att @ v per quarter, then per-slot finalize ----
                for qtr in range(4):
                    with nc.named_scope(f"av{qtr}"):
                        vq = [pb.tile([128, 1024], bf16, tag=f"vq{j}", bufs=2, name=f"vq{j}") for j in range(8)]
                        for j in range(8):
                            nc.sync.dma_start(
                                out=vq[j][:],
                                in_=vag_d[1024 * qtr + 128 * j:1024 * qtr + 128 * (j + 1), :],
                            )
                        for s in range(4):
                            if qtr >= PADQ[s]:
                                continue
                            last_q = (qtr == PADQ[s] - 1)
                            pt_sb = pt_tiles[(s, qtr)]
                            ps_av = psp.tile([128, 1024], f32, tag="ps_av", name="ps_av", bufs=2)
                            for h in range(2):
                                for j in range(8):
                                    nc.tensor.matmul(
                                        ps_av[:, 512 * h:512 * (h + 1)],
                                        pt_sb[:, 128 * j:128 * (j + 1)],
                                        vq[j][:, 512 * h:512 * (h + 1)],
                                        start=(j == 0), stop=(j == 7),
                                    )
                            if qtr == 0:
                                nc.vector.tensor_copy(A_sb[s][:], ps_av[:])
                            else:
                                nc.vector.tensor_tensor(
                                    out=A_sb[s][:], in0=A_sb[s][:], in1=ps_av[:], op=OP.add)

                            # ---------- finalize slot after its last quarter ----------
                            if last_q:
                                with nc.named_scope(f"fin{s}"):
                                    lsum = pb.tile([128, 1], f32, tag="lsum", bufs=2, name="lsum")
                                    if PADQ[s] > 1:
                                        nc.vector.reduce_sum(
                                            lsum[:], lpart[s][:, 0:PADQ[s]], axis=AX.X)
                                    else:
                                        nc.vector.tensor_copy(lsum[:], lpart[s][:, 0:1])
                                    rl = pb.tile([128, 1], f32, tag="rl", bufs=2, name="rl")
                                    nc.vector.reciprocal(rl[:], lsum[:])
                                    attn_f = pb.tile([128, D], f32, tag="attn_f", bufs=2, name="attn_f")
                                    nc.scalar.activation(
                                        attn_f[:], A_sb[s][:], AT.Copy, bias=0.0,
                                        scale=rl[:, 0:1])
                                    nc.sync.dma_start(
                                        out=attn_d[128 * s:128 * (s + 1), :], in_=attn_f[:])
                                    attn_b = pb.tile([128, D], bf16, tag="attn_b", bufs=2, name="attn_b")
                                    nc.vector.tensor_copy(attn_b[:], attn_f[:])
                                    ps_t2 = psp.tile([128, 1024], bf16, tag="ps_trrow", bufs=1, name="ps_trrow")
                                    for ec in range(8):
                                        nc.tensor.transpose(
                                            ps_t2[:, 128 * ec:128 * (ec + 1)],
                                            attn_b[:, 128 * ec:128 * (ec + 1)],
                                            ident[:])
                                    at_row = pb.tile([128, 1024], bf16, tag="at_sb", bufs=2, name="at_sb")
                                    nc.scalar.copy(at_row[:], ps_t2[:])
                                    xqr = pb.tile([128, D], f32, tag="xqr", bufs=2, name="xqr")
                                    nc.sync.dma_start(
                                        out=xqr[:], in_=xqres_d[128 * s:128 * (s + 1), :])
                                    out_sb = pb.tile([128, D], f32, tag="out_sb", bufs=2, name="out_sb")
                                    for h in range(2):
                                        ps_o = psp.tile([128, 512], f32, tag="pp", name="pp", bufs=3)
                                        for ec in range(8):
                                            nc.tensor.matmul(
                                                ps_o[:], at_row[:, 128 * ec:128 * (ec + 1)],
                                                wproj[ec][:, 512 * h:512 * (h + 1)],
                                                start=(ec == 0), stop=(ec == 7),
                                            )
                                        nc.vector.tensor_tensor(
                                            out=out_sb[:, 512 * h:512 * (h + 1)], in0=ps_o[:],
                                            in1=xqr[:, 512 * h:512 * (h + 1)], op=OP.add)
                                    nc.sync.dma_start(
                                        out=out_d[128 * s:128 * (s + 1), :], in_=out_sb[:])

    nc.compile()
    return nc


def _get_compiled():
    global _COMPILED
    if _COMPILED is None:
        _COMPILED = _build()
    return _COMPILED


def kernel(x, attention_mask, Wq, Wkv, Wproj, _trace=False):
    global LAST_EXEC_NS
    from concourse.bass_utils import run_bass_kernel_spmd

    x = np.asarray(x)
    attention_mask = np.asarray(attention_mask)
    Wq, Wkv, Wproj = np.asarray(Wq), np.asarray(Wkv), np.asarray(Wproj)
    assert x.shape == (T, D) and attention_mask.shape == (T,)
    assert np.array_equal(attention_mask, np.arange(T, dtype=attention_mask.dtype)), \
        "kernel assumes attention_mask == arange(T)"

    x16 = x.astype(np.float16)
    wqT = np.ascontiguousarray(Wq.T).astype(np.float16)
    wkT = np.ascontiguousarray(Wkv[:D].T).astype(np.float16)
    wvT = np.ascontiguousarray(Wkv[D:].T).astype(np.float16)
    wpT = np.ascontiguousarray(Wproj.T).astype(ml_dtypes.bfloat16)

    in_maps = []
    core_rows = []
    for c in range(N_CORES):
        blocks = core_blocks(c)
        rows = np.concatenate([np.arange(128 * b, 128 * (b + 1)) for b in blocks])
        core_rows.append(rows)
        xq = x[rows]                      # [512, D] f32
        xqT = np.ascontiguousarray(x16[rows].T)    # [D, 512] f16
        xkvT = np.ascontiguousarray(x16[512 * c:512 * (c + 1)].T)
        pos = np.empty((128, 4), np.float32)
        for s, b in enumerate(blocks):
            pos[:, s] = 128 * b + np.arange(128)
        in_maps.append({
            "xqT": xqT, "xkvT": xkvT,
            "xqres": np.ascontiguousarray(xq.astype(np.float32)),
            "pos": pos,
            "wqT": wqT, "wkT": wkT, "wvT": wvT, "wpT": wpT,
        })

    nc = _get_compiled()
    res = run_bass_kernel_spmd(nc, in_maps, list(range(N_CORES)), trace=_trace)
    LAST_EXEC_NS = res.exec_time_ns
    globals()["LAST_RES"] = res

    out_full = np.empty((T, D), np.float32)
    x_new = x.astype(np.float32).copy()
    for c in range(N_CORES):
        r = res.results[c]
        out_full[core_rows[c]] = r["out"]
        x_new[core_rows[c]] += r["attn"]
    return out_full, x_new



# revision 15
# speedup vs baseline: 1.1046x; 1.0447x over previous
"""Trainium2 Bass kernel for nn_CharAttention (causal single-head attention, T=4096, D=1024).

Strategy (8 NeuronCores, SPMD):
  - Queries sharded across cores with a balanced causal interleave: core c owns
    global 128-row q-blocks {c, 15-c, 16+c, 31-c} ("slots" 0..3), so every core
    does the same causal work (structurally identical static program).
  - k/v computed shard-wise (core c projects rows [512c, 512c+512)) then
    AllGather'd in fp16/bf16.
  - Slot s is padded to (s+1)*1024 key-columns; the data-dependent causal
    boundary is applied with an iota>pos additive -1e9 mask on the last quarter
    of each slot (the diagonal always lands there for every core).
  - Softmax without a running max: m_hat = rowmax(first 1024 cols) + 50.
    exp(s - m_hat) stays within bf16/f32 range (margin analysis: overflow needs
    a later-quarter score 138 above the quarter-0 max; underfow drops only
    weights < 1e-16 of the total), so quarter contributions accumulate with
    plain adds and one final 1/l normalization.
  - dtypes: q/k/scores chain in fp16 (PE full rate, 8x finer mantissa than
    bf16 -- scores have std ~32 and softmax is argmax-sensitive); p/v/attn/proj
    in bf16 (needs fp32-wide exponent range for the shifted exp).
"""

import numpy as np
import ml_dtypes

T = 4096
D = 1024
N_CORES = 8
NBLK = T // 128  # 32 global q-blocks
DELTA = 50.0
NEG_BIG = -1e9

# slot assignment: core c -> global blocks [c, 15-c, 16+c, 31-c]
def core_blocks(c):
    return [c, 15 - c, 16 + c, 31 - c]

PADQ = [1, 2, 3, 4]  # quarters (1024 cols) computed per slot

FILL_AGK = 0  # filler matmuls (~213ns each) bridging the AG(k) wait
FILL_AGV = 0  # filler matmuls bridging the AG(v) wait

_COMPILED = None
LAST_EXEC_NS = None


def _build():
    import concourse.bass as bass
    import concourse.mybir as mybir
    from concourse import bacc
    from concourse.tile import TileContext
    from concourse.masks import make_identity

    f16, bf16, f32 = mybir.dt.float16, mybir.dt.bfloat16, mybir.dt.float32
    AT = mybir.ActivationFunctionType
    OP = mybir.AluOpType
    AX = mybir.AxisListType

    nc = bacc.Bacc("TRN2", target_bir_lowering=False, debug=False, num_devices=N_CORES)

    # --- I/O ---
    xqT_d = nc.dram_tensor("xqT", [D, 512], f16, kind="ExternalInput")
    xkvT_d = nc.dram_tensor("xkvT", [D, 512], f16, kind="ExternalInput")
    xqres_d = nc.dram_tensor("xqres", [512, D], f32, kind="ExternalInput")
    pos_d = nc.dram_tensor("pos", [128, 4], f32, kind="ExternalInput")
    wqT_d = nc.dram_tensor("wqT", [D, D], f16, kind="ExternalInput")
    wkT_d = nc.dram_tensor("wkT", [D, D], f16, kind="ExternalInput")
    wvT_d = nc.dram_tensor("wvT", [D, D], f16, kind="ExternalInput")
    wpT_d = nc.dram_tensor("wpT", [D, D], bf16, kind="ExternalInput")
    out_d = nc.dram_tensor("out", [512, D], f32, kind="ExternalOutput")
    attn_d = nc.dram_tensor("attn", [512, D], f32, kind="ExternalOutput")

    # internal DRAM for the collective
    ktloc_d = nc.dram_tensor("ktloc", [D, 512], f16)
    vloc_d = nc.dram_tensor("vloc", [512, D], bf16)
    ktag_d = nc.dram_tensor("ktag", [N_CORES, D, 512], f16, addr_space="Shared")
    vag_d = nc.dram_tensor("vag", [T, D], bf16, addr_space="Shared")
    dumloc_d = nc.dram_tensor("dumloc", [128, 4], f32)
    dumag_d = nc.dram_tensor("dumag", [N_CORES, 128, 4], f32, addr_space="Shared")

    groups = [list(range(N_CORES))]

    with TileContext(nc) as tc:
        with (
            tc.tile_pool(name="persist", bufs=1) as pp,
            tc.tile_pool(name="psum", bufs=1, space="PSUM") as psp,
        ):
            # persistent tiles
            warm_sb = pp.tile([128, 512], f16, tag="warm_sb", name="warm_sb")
            nc.vector.memset(warm_sb[:], 0.0)
            # HAM warmup: real matmuls (transposes don't count as PE-busy for
            # HAM) with no DMA deps, so the PE reaches K=8/8 before k_proj and
            # stays busy during the input-DMA lead-in.
            for w in range(6):
                wps = psp.tile([128, 512], f32, tag="pp", name="pp", bufs=3)
                nc.tensor.matmul(wps[:], warm_sb[:, 0:128], warm_sb[:], start=True, stop=True)

            qt = [pp.tile([128, 512], f16, tag=f"qt{e}", name=f"qt{e}") for e in range(8)]
            ident = pp.tile([128, 128], bf16, tag="ident", name="ident")
            make_identity(nc, ident[:])
            iota_i = pp.tile([128, 512], mybir.dt.int32, tag="iota_i", name="iota_i")
            nc.gpsimd.iota(iota_i[:], pattern=[[1, 512]], base=0, channel_multiplier=0)
            iota_f = pp.tile([128, 512], f32, tag="iota_f", name="iota_f")
            nc.vector.tensor_copy(iota_f[:], iota_i[:])
            pos_sb = pp.tile([128, 4], f32, tag="pos_sb", name="pos_sb")
            nc.sync.dma_start(out=pos_sb[:], in_=pos_d[:])
            A_sb = [pp.tile([128, D], f32, tag=f"A{s}", name=f"A{s}") for s in range(4)]
            lpart = [pp.tile([128, 4], f32, tag=f"lp{s}", name=f"lp{s}") for s in range(4)]
            negm = [pp.tile([128, 1], f32, tag=f"nm{s}", name=f"nm{s}") for s in range(4)]

            # dummy collective: absorbs the ~17us collective arming/sync cost
            # before the real AllGathers are triggered
            dum_sb = pp.tile([128, 4], f32, tag="dum_sb", name="dum_sb")
            nc.vector.memset(dum_sb[:], 0.0)
            nc.sync.dma_start(out=dumloc_d[:], in_=dum_sb[:])
            nc.gpsimd.collective_compute(
                "AllGather", mybir.AluOpType.bypass, replica_groups=groups,
                ins=[dumloc_d[:]], outs=[dumag_d[:]],
            )

            # ---------------- Phase A: projections + allgather ----------------
            with tc.tile_pool(name="pha", bufs=1) as pa:
                # per-chunk loads, dispatch spread across engine queues
                # (sync: xkvt, scalar: wk, vector: wv, gpsimd: xq/wq)
                xkvt = [pa.tile([128, 512], f16, tag=f"xkvt{d}", name=f"xkvt{d}") for d in range(8)]
                wk = [pa.tile([128, D], f16, tag=f"wk{d}", name=f"wk{d}") for d in range(8)]
                wv = [pa.tile([128, D], f16, tag=f"wv{d}", name=f"wv{d}") for d in range(8)]
                xqt = [pa.tile([128, 512], f16, tag=f"xqt{d}", name=f"xqt{d}") for d in range(8)]
                wq = [pa.tile([128, D], f16, tag=f"wq{d}", name=f"wq{d}") for d in range(8)]
                for d in range(8):
                    nc.sync.dma_start(out=xkvt[d][:], in_=xkvT_d[128 * d:128 * (d + 1), :])
                    nc.scalar.dma_start(out=wk[d][:], in_=wkT_d[128 * d:128 * (d + 1), :])
                for d in range(8):
                    nc.gpsimd.dma_start(out=wv[d][:], in_=wvT_d[128 * d:128 * (d + 1), :])
                for d in range(8):
                    nc.gpsimd.dma_start(out=xqt[d][:], in_=xqT_d[128 * d:128 * (d + 1), :])
                    nc.gpsimd.dma_start(out=wq[d][:], in_=wqT_d[128 * d:128 * (d + 1), :])

                with nc.named_scope("k_proj"):
                    # kT_local[e,t] = sum_d WkT[d,e] * xkvT[d,t]
                    for e in range(8):
                        ps = psp.tile([128, 512], f32, tag="pp", name="pp", bufs=3)
                        for d in range(8):
                            nc.tensor.matmul(
                                ps[:], wk[d][:, 128 * e:128 * (e + 1)], xkvt[d][:],
                                start=(d == 0), stop=(d == 7),
                            )
                        kt_sb = pa.tile([128, 512], f16, tag="kt_sb", name="kt_sb", bufs=3)
                        nc.scalar.copy(kt_sb[:], ps[:])
                        nc.sync.dma_start(out=ktloc_d[128 * e:128 * (e + 1), :], in_=kt_sb[:])

                nc.gpsimd.collective_compute(
                    "AllGather", mybir.AluOpType.bypass, replica_groups=groups,
                    ins=[ktloc_d[:]], outs=[ktag_d[:]],
                )

                with nc.named_scope("v_proj"):
                    # v_local[t,e] = sum_d xkvT[d,t] * WvT[d,e]
                    for t in range(4):
                        for h in range(2):
                            ps = psp.tile([128, 512], f32, tag="pp", name="pp", bufs=3)
                            for d in range(8):
                                nc.tensor.matmul(
                                    ps[:], xkvt[d][:, 128 * t:128 * (t + 1)],
                                    wv[d][:, 512 * h:512 * (h + 1)],
                                    start=(d == 0), stop=(d == 7),
                                )
                            v_sb = pa.tile([128, 512], bf16, tag="v_sb", name="v_sb", bufs=3)
                            nc.scalar.copy(v_sb[:], ps[:])
                            nc.sync.dma_start(
                                out=vloc_d[128 * t:128 * (t + 1), 512 * h:512 * (h + 1)],
                                in_=v_sb[:],
                            )

                nc.gpsimd.collective_compute(
                    "AllGather", mybir.AluOpType.bypass, replica_groups=groups,
                    ins=[vloc_d[:]], outs=[vag_d[:]],
                )

                with nc.named_scope("q_proj"):
                    for e in range(8):
                        ps = psp.tile([128, 512], f32, tag="pp", name="pp", bufs=3)
                        for d in range(8):
                            nc.tensor.matmul(
                                ps[:], wq[d][:, 128 * e:128 * (e + 1)], xqt[d][:],
                                start=(d == 0), stop=(d == 7),
                            )
                        nc.scalar.copy(qt[e][:], ps[:])

            # ---------------- Phase B: attention over quarters ----------------
            with tc.tile_pool(name="phb", bufs=1) as pb:
                wproj = [pb.tile([128, D], bf16, tag=f"wp{d}", name=f"wp{d}") for d in range(8)]
                for d in range(8):
                    nc.sync.dma_start(out=wproj[d][:], in_=wpT_d[128 * d:128 * (d + 1), :])

                # ---- Pass 1: all scores + exp + P-transposes (overlaps v-AllGather) ----
                pt_tiles = {}
                for qtr in range(4):
                    with nc.named_scope(f"sc{qtr}"):
                        kq = [pb.tile([128, 1024], f16, tag=f"kq{e}", bufs=4, name=f"kq{e}") for e in range(8)]
                        for e in range(8):
                            eng = nc.sync if e % 2 == 0 else nc.gpsimd
                            for hh in range(2):
                                eng.dma_start(
                                    out=kq[e][:, 512 * hh:512 * (hh + 1)],
                                    in_=ktag_d[2 * qtr + hh, 128 * e:128 * (e + 1), :],
                                )
                        for s in range(4):
                            if qtr >= PADQ[s]:
                                continue
                            last_q = (qtr == PADQ[s] - 1)
                            ssb = pb.tile([128, 1024], f32, tag="ssb", bufs=2, name="ssb")
                            for pn in range(2):
                                ps = psp.tile([128, 512], f32, tag="pp", name="pp", bufs=3)
                                for e in range(8):
                                    nc.tensor.matmul(
                                        ps[:], qt[e][:, 128 * s:128 * (s + 1)],
                                        kq[e][:, 512 * pn:512 * (pn + 1)],
                                        start=(e == 0), stop=(e == 7),
                                    )
                                if last_q:
                                    shift = pb.tile([128, 1], f32, tag="shift", bufs=2, name="shift")
                                    nc.vector.tensor_scalar_add(
                                        shift[:], pos_sb[:, s:s + 1],
                                        float(-(qtr * 1024 + pn * 512)),
                                    )
                                    madd = pb.tile([128, 512], f32, tag="madd", bufs=1, name="madd")
                                    nc.vector.tensor_scalar(
                                        out=madd[:], in0=iota_f[:], scalar1=shift[:, 0:1],
                                        scalar2=NEG_BIG, op0=OP.is_gt, op1=OP.mult,
                                    )
                                    nc.vector.tensor_tensor(
                                        out=ssb[:, 512 * pn:512 * (pn + 1)], in0=ps[:],
                                        in1=madd[:], op=OP.add,
                                    )
                                else:
                                    nc.scalar.copy(ssb[:, 512 * pn:512 * (pn + 1)], ps[:])
                            if qtr == 0:
                                nc.vector.reduce_max(
                                    negm[s][:], ssb[:], axis=AX.X, negate=True)
                                nc.vector.tensor_scalar_add(negm[s][:], negm[s][:], -DELTA)
                            psb = pb.tile([128, 1024], bf16, tag="psb", bufs=3, name="psb")
                            nc.scalar.activation(
                                psb[:], ssb[:], AT.Exp, bias=negm[s][:, 0:1], scale=1.0,
                                accum_out=lpart[s][:, qtr:qtr + 1],
                            )
                            ps_tr = psp.tile([128, 1024], bf16, tag="ps_trrow", bufs=1, name="ps_trrow")
                            for j in range(8):
                                nc.tensor.transpose(
                                    ps_tr[:, 128 * j:128 * (j + 1)],
                                    psb[:, 128 * j:128 * (j + 1)], ident[:])
                            pt_sb = pb.tile([128, 1024], bf16, tag="pt_sb", bufs=9, name="pt_sb")
                            nc.scalar.copy(pt_sb[:], ps_tr[:])
                            pt_tiles[(s, qtr)] = pt_sb

                # ---- Pass 2: att @ v per quarter, then per-slot finalize ----
                for qtr in range(4):
                    with nc.named_scope(f"av{qtr}"):
                        vq = [pb.tile([128, 1024], bf16, tag=f"vq{j}", bufs=2, name=f"vq{j}") for j in range(8)]
                        for j in range(8):
                            eng = nc.sync if j % 2 == 0 else nc.gpsimd
                            eng.dma_start(
                                out=vq[j][:],
                                in_=vag_d[1024 * qtr + 128 * j:1024 * qtr + 128 * (j + 1), :],
                            )
                        for s in range(4):
                            if qtr >= PADQ[s]:
                                continue
                            last_q = (qtr == PADQ[s] - 1)
                            pt_sb = pt_tiles[(s, qtr)]
                            ps_av = psp.tile([128, 1024], f32, tag="ps_av", name="ps_av", bufs=2)
                            for h in range(2):
                                for j in range(8):
                                    nc.tensor.matmul(
                                        ps_av[:, 512 * h:512 * (h + 1)],
                                        pt_sb[:, 128 * j:128 * (j + 1)],
                                        vq[j][:, 512 * h:512 * (h + 1)],
                                        start=(j == 0), stop=(j == 7),
                                    )
                            if qtr == 0:
                                nc.vector.tensor_copy(A_sb[s][:], ps_av[:])
                            else:
                                nc.vector.tensor_tensor(
                                    out=A_sb[s][:], in0=A_sb[s][:], in1=ps_av[:], op=OP.add)

                            # ---------- finalize slot after its last quarter ----------
                            if last_q:
                                with nc.named_scope(f"fin{s}"):
                                    lsum = pb.tile([128, 1], f32, tag="lsum", bufs=2, name="lsum")
                                    if PADQ[s] > 1:
                                        nc.vector.reduce_sum(
                                            lsum[:], lpart[s][:, 0:PADQ[s]], axis=AX.X)
                                    else:
                                        nc.vector.tensor_copy(lsum[:], lpart[s][:, 0:1])
                                    rl = pb.tile([128, 1], f32, tag="rl", bufs=2, name="rl")
                                    nc.vector.reciprocal(rl[:], lsum[:])
                                    attn_f = pb.tile([128, D], f32, tag="attn_f", bufs=2, name="attn_f")
                                    nc.scalar.activation(
                                        attn_f[:], A_sb[s][:], AT.Copy, bias=0.0,
                                        scale=rl[:, 0:1])
                                    nc.sync.dma_start(
                                        out=attn_d[128 * s:128 * (s + 1), :], in_=attn_f[:])
                                    attn_b = pb.tile([128, D], bf16, tag="attn_b", bufs=2, name="attn_b")
                                    nc.vector.tensor_copy(attn_b[:], attn_f[:])
                                    ps_t2 = psp.tile([128, 1024], bf16, tag="ps_trrow", bufs=1, name="ps_trrow")
                                    for ec in range(8):
                                        nc.tensor.transpose(
                                            ps_t2[:, 128 * ec:128 * (ec + 1)],
                                            attn_b[:, 128 * ec:128 * (ec + 1)],
                                            ident[:])
                                    at_row = pb.tile([128, 1024], bf16, tag="at_sb", bufs=2, name="at_sb")
                                    nc.scalar.copy(at_row[:], ps_t2[:])
                                    xqr = pb.tile([128, D], f32, tag="xqr", bufs=2, name="xqr")
                                    nc.sync.dma_start(
                                        out=xqr[:], in_=xqres_d[128 * s:128 * (s + 1), :])
                                    out_sb = pb.tile([128, D], f32, tag="out_sb", bufs=2, name="out_sb")
                                    for h in range(2):
                                        ps_o = psp.tile([128, 512], f32, tag="pp", name="pp", bufs=3)
                                        for ec in range(8):
                                            nc.tensor.matmul(
                                                ps_o[:], at_row[:, 128 * ec:128 * (ec + 1)],
                                                wproj[ec][:, 512 * h:512 * (h + 1)],
                                                start=(ec == 0), stop=(ec == 7),
                                            )
                                        nc.vector.tensor_tensor(
                                            out=out_sb[:, 512 * h:512 * (h + 1)], in0=ps_o[:],
                                            in1=xqr[:, 512 * h:512 * (h + 1)], op=OP.add)
                                    nc.sync.dma_start(
                                        out=out_d[128 * s:128 * (s + 1), :], in_=out_sb[:])

    nc.compile()
    return nc


def _get_compiled():
    global _COMPILED
    if _COMPILED is None:
        _COMPILED = _build()
    return _COMPILED


def kernel(x, attention_mask, Wq, Wkv, Wproj, _trace=False):
    global LAST_EXEC_NS
    from concourse.bass_utils import run_bass_kernel_spmd

    x = np.asarray(x)
    attention_mask = np.asarray(attention_mask)
    Wq, Wkv, Wproj = np.asarray(Wq), np.asarray(Wkv), np.asarray(Wproj)
    assert x.shape == (T, D) and attention_mask.shape == (T,)
    assert np.array_equal(attention_mask, np.arange(T, dtype=attention_mask.dtype)), \
        "kernel assumes attention_mask == arange(T)"

    x16 = x.astype(np.float16)
    wqT = np.ascontiguousarray(Wq.T).astype(np.float16)
    wkT = np.ascontiguousarray(Wkv[:D].T).astype(np.float16)
    wvT = np.ascontiguousarray(Wkv[D:].T).astype(np.float16)
    wpT = np.ascontiguousarray(Wproj.T).astype(ml_dtypes.bfloat16)

    in_maps = []
    core_rows = []
    for c in range(N_CORES):
        blocks = core_blocks(c)
        rows = np.concatenate([np.arange(128 * b, 128 * (b + 1)) for b in blocks])
        core_rows.append(rows)
        xq = x[rows]                      # [512, D] f32
        xqT = np.ascontiguousarray(x16[rows].T)    # [D, 512] f16
        xkvT = np.ascontiguousarray(x16[512 * c:512 * (c + 1)].T)
        pos = np.empty((128, 4), np.float32)
        for s, b in enumerate(blocks):
            pos[:, s] = 128 * b + np.arange(128)
        in_maps.append({
            "xqT": xqT, "xkvT": xkvT,
            "xqres": np.ascontiguousarray(xq.astype(np.float32)),
            "pos": pos,
            "wqT": wqT, "wkT": wkT, "wvT": wvT, "wpT": wpT,
        })

    nc = _get_compiled()
    res = run_bass_kernel_spmd(nc, in_maps, list(range(N_CORES)), trace=_trace)
    LAST_EXEC_NS = res.exec_time_ns
    globals()["LAST_RES"] = res

    out_full = np.empty((T, D), np.float32)
    x_new = x.astype(np.float32).copy()
    for c in range(N_CORES):
        r = res.results[c]
        out_full[core_rows[c]] = r["out"]
        x_new[core_rows[c]] += r["attn"]
    return out_full, x_new



# revision 16
# speedup vs baseline: 1.1581x; 1.0484x over previous
"""Trainium2 Bass kernel for nn_CharAttention (causal single-head attention, T=4096, D=1024).

Strategy (8 NeuronCores, SPMD):
  - Queries sharded across cores with a balanced causal interleave: core c owns
    global 128-row q-blocks {c, 15-c, 16+c, 31-c} ("slots" 0..3), so every core
    does the same causal work (structurally identical static program).
  - k/v computed shard-wise (core c projects rows [512c, 512c+512)) then
    AllGather'd in fp16/bf16.
  - Slot s is padded to (s+1)*1024 key-columns; the data-dependent causal
    boundary is applied with an iota>pos additive -1e9 mask on the last quarter
    of each slot (the diagonal always lands there for every core).
  - Softmax without a running max: m_hat = rowmax(first 1024 cols) + 50.
    exp(s - m_hat) stays within bf16/f32 range (margin analysis: overflow needs
    a later-quarter score 138 above the quarter-0 max; underfow drops only
    weights < 1e-16 of the total), so quarter contributions accumulate with
    plain adds and one final 1/l normalization.
  - dtypes: q/k/scores chain in fp16 (PE full rate, 8x finer mantissa than
    bf16 -- scores have std ~32 and softmax is argmax-sensitive); p/v/attn/proj
    in bf16 (needs fp32-wide exponent range for the shifted exp).
"""

import numpy as np
import ml_dtypes

T = 4096
D = 1024
N_CORES = 8
NBLK = T // 128  # 32 global q-blocks
DELTA = 50.0
NEG_BIG = -1e9

# slot assignment: core c -> global blocks [c, 15-c, 16+c, 31-c]
def core_blocks(c):
    return [c, 15 - c, 16 + c, 31 - c]

PADQ = [1, 2, 3, 4]  # quarters (1024 cols) computed per slot

FILL_AGK = 0  # filler matmuls (~213ns each) bridging the AG(k) wait
FILL_AGV = 0  # filler matmuls bridging the AG(v) wait

_COMPILED = None
LAST_EXEC_NS = None


def _build():
    import concourse.bass as bass
    import concourse.mybir as mybir
    from concourse import bacc
    from concourse.tile import TileContext
    from concourse.masks import make_identity

    f16, bf16, f32 = mybir.dt.float16, mybir.dt.bfloat16, mybir.dt.float32
    AT = mybir.ActivationFunctionType
    OP = mybir.AluOpType
    AX = mybir.AxisListType

    nc = bacc.Bacc("TRN2", target_bir_lowering=False, debug=False, num_devices=N_CORES)

    # --- I/O ---
    xqT_d = nc.dram_tensor("xqT", [D, 512], f16, kind="ExternalInput")
    xkvT_d = nc.dram_tensor("xkvT", [D, 512], f16, kind="ExternalInput")
    xqres_d = nc.dram_tensor("xqres", [512, D], f32, kind="ExternalInput")
    pos_d = nc.dram_tensor("pos", [128, 4], f32, kind="ExternalInput")
    wqT_d = nc.dram_tensor("wqT", [D, D], f16, kind="ExternalInput")
    wkT_d = nc.dram_tensor("wkT", [D, D], f16, kind="ExternalInput")
    wvT_d = nc.dram_tensor("wvT", [D, D], f16, kind="ExternalInput")
    wpT_d = nc.dram_tensor("wpT", [D, D], bf16, kind="ExternalInput")
    out_d = nc.dram_tensor("out", [512, D], f32, kind="ExternalOutput")
    attn_d = nc.dram_tensor("attn", [512, D], f32, kind="ExternalOutput")

    # internal DRAM for the collective
    ktloc_d = nc.dram_tensor("ktloc", [D, 512], f16)
    vloc_d = nc.dram_tensor("vloc", [512, D], bf16)
    ktag_d = nc.dram_tensor("ktag", [N_CORES, D, 512], f16, addr_space="Shared")
    vag_d = nc.dram_tensor("vag", [T, D], bf16, addr_space="Shared")
    dumloc_d = nc.dram_tensor("dumloc", [128, 4], f32)
    dumag_d = nc.dram_tensor("dumag", [N_CORES, 128, 4], f32, addr_space="Shared")

    groups = [list(range(N_CORES))]

    with TileContext(nc) as tc:
        with (
            tc.tile_pool(name="persist", bufs=1) as pp,
            tc.tile_pool(name="psum", bufs=1, space="PSUM") as psp,
        ):
            # persistent tiles
            warm_sb = pp.tile([128, 512], f16, tag="warm_sb", name="warm_sb")
            nc.vector.memset(warm_sb[:], 0.0)
            # HAM warmup: real matmuls (transposes don't count as PE-busy for
            # HAM) with no DMA deps, so the PE reaches K=8/8 before k_proj and
            # stays busy during the input-DMA lead-in.
            for w in range(6):
                wps = psp.tile([128, 512], f32, tag="pp", name="pp", bufs=3)
                nc.tensor.matmul(wps[:], warm_sb[:, 0:128], warm_sb[:], start=True, stop=True)

            qt = [pp.tile([128, 512], f16, tag=f"qt{e}", name=f"qt{e}") for e in range(8)]
            ident = pp.tile([128, 128], bf16, tag="ident", name="ident")
            make_identity(nc, ident[:])
            iota_i = pp.tile([128, 512], mybir.dt.int32, tag="iota_i", name="iota_i")
            nc.gpsimd.iota(iota_i[:], pattern=[[1, 512]], base=0, channel_multiplier=0)
            iota_f = pp.tile([128, 512], f32, tag="iota_f", name="iota_f")
            nc.vector.tensor_copy(iota_f[:], iota_i[:])
            pos_sb = pp.tile([128, 4], f32, tag="pos_sb", name="pos_sb")
            nc.sync.dma_start(out=pos_sb[:], in_=pos_d[:])
            A_sb = [pp.tile([128, D], f32, tag=f"A{s}", name=f"A{s}") for s in range(4)]
            lpart = [pp.tile([128, 4], f32, tag=f"lp{s}", name=f"lp{s}") for s in range(4)]
            negm = [pp.tile([128, 1], f32, tag=f"nm{s}", name=f"nm{s}") for s in range(4)]

            # ---------------- Phase A: projections + allgather ----------------
            with tc.tile_pool(name="pha", bufs=1) as pa:
                # per-chunk loads, dispatch spread across engine queues
                # (sync: xkvt, scalar: wk, vector: wv, gpsimd: xq/wq)
                xkvt = [pa.tile([128, 512], f16, tag=f"xkvt{d}", name=f"xkvt{d}") for d in range(8)]
                wk = [pa.tile([128, D], f16, tag=f"wk{d}", name=f"wk{d}") for d in range(8)]
                wv = [pa.tile([128, D], f16, tag=f"wv{d}", name=f"wv{d}") for d in range(8)]
                xqt = [pa.tile([128, 512], f16, tag=f"xqt{d}", name=f"xqt{d}") for d in range(8)]
                wq = [pa.tile([128, D], f16, tag=f"wq{d}", name=f"wq{d}") for d in range(8)]
                for d in range(8):
                    nc.sync.dma_start(out=xkvt[d][:], in_=xkvT_d[128 * d:128 * (d + 1), :])
                    nc.scalar.dma_start(out=wk[d][:], in_=wkT_d[128 * d:128 * (d + 1), :])
                for d in range(8):
                    nc.gpsimd.dma_start(out=wv[d][:], in_=wvT_d[128 * d:128 * (d + 1), :])
                for d in range(8):
                    nc.gpsimd.dma_start(out=xqt[d][:], in_=xqT_d[128 * d:128 * (d + 1), :])
                    nc.gpsimd.dma_start(out=wq[d][:], in_=wqT_d[128 * d:128 * (d + 1), :])

                with nc.named_scope("k_proj"):
                    # kT_local[e,t] = sum_d WkT[d,e] * xkvT[d,t]
                    for e in range(8):
                        ps = psp.tile([128, 512], f32, tag="pp", name="pp", bufs=3)
                        for d in range(8):
                            nc.tensor.matmul(
                                ps[:], wk[d][:, 128 * e:128 * (e + 1)], xkvt[d][:],
                                start=(d == 0), stop=(d == 7),
                            )
                        kt_sb = pa.tile([128, 512], f16, tag="kt_sb", name="kt_sb", bufs=3)
                        nc.scalar.copy(kt_sb[:], ps[:])
                        nc.sync.dma_start(out=ktloc_d[128 * e:128 * (e + 1), :], in_=kt_sb[:])

                nc.gpsimd.collective_compute(
                    "AllGather", mybir.AluOpType.bypass, replica_groups=groups,
                    ins=[ktloc_d[:]], outs=[ktag_d[:]],
                )

                with nc.named_scope("v_proj"):
                    # v_local[t,e] = sum_d xkvT[d,t] * WvT[d,e]
                    for t in range(4):
                        for h in range(2):
                            ps = psp.tile([128, 512], f32, tag="pp", name="pp", bufs=3)
                            for d in range(8):
                                nc.tensor.matmul(
                                    ps[:], xkvt[d][:, 128 * t:128 * (t + 1)],
                                    wv[d][:, 512 * h:512 * (h + 1)],
                                    start=(d == 0), stop=(d == 7),
                                )
                            v_sb = pa.tile([128, 512], bf16, tag="v_sb", name="v_sb", bufs=3)
                            nc.scalar.copy(v_sb[:], ps[:])
                            nc.sync.dma_start(
                                out=vloc_d[128 * t:128 * (t + 1), 512 * h:512 * (h + 1)],
                                in_=v_sb[:],
                            )

                nc.gpsimd.collective_compute(
                    "AllGather", mybir.AluOpType.bypass, replica_groups=groups,
                    ins=[vloc_d[:]], outs=[vag_d[:]],
                )

                with nc.named_scope("q_proj"):
                    for e in range(8):
                        ps = psp.tile([128, 512], f32, tag="pp", name="pp", bufs=3)
                        for d in range(8):
                            nc.tensor.matmul(
                                ps[:], wq[d][:, 128 * e:128 * (e + 1)], xqt[d][:],
                                start=(d == 0), stop=(d == 7),
                            )
                        nc.scalar.copy(qt[e][:], ps[:])

            # ---------------- Phase B: attention over quarters ----------------
            with tc.tile_pool(name="phb", bufs=1) as pb:
                wproj = [pb.tile([128, D], bf16, tag=f"wp{d}", name=f"wp{d}") for d in range(8)]
                for d in range(8):
                    nc.sync.dma_start(out=wproj[d][:], in_=wpT_d[128 * d:128 * (d + 1), :])

                # ---- Pass 1: all scores + exp + P-transposes (overlaps v-AllGather) ----
                pt_tiles = {}
                for qtr in range(4):
                    with nc.named_scope(f"sc{qtr}"):
                        kq = [pb.tile([128, 1024], f16, tag=f"kq{e}", bufs=4, name=f"kq{e}") for e in range(8)]
                        for e in range(8):
                            eng = nc.sync if e % 2 == 0 else nc.gpsimd
                            for hh in range(2):
                                eng.dma_start(
                                    out=kq[e][:, 512 * hh:512 * (hh + 1)],
                                    in_=ktag_d[2 * qtr + hh, 128 * e:128 * (e + 1), :],
                                )
                        for s in range(4):
                            if qtr >= PADQ[s]:
                                continue
                            last_q = (qtr == PADQ[s] - 1)
                            ssb = pb.tile([128, 1024], f32, tag="ssb", bufs=2, name="ssb")
                            for pn in range(2):
                                ps = psp.tile([128, 512], f32, tag="pp", name="pp", bufs=3)
                                for e in range(8):
                                    nc.tensor.matmul(
                                        ps[:], qt[e][:, 128 * s:128 * (s + 1)],
                                        kq[e][:, 512 * pn:512 * (pn + 1)],
                                        start=(e == 0), stop=(e == 7),
                                    )
                                if last_q:
                                    shift = pb.tile([128, 1], f32, tag="shift", bufs=2, name="shift")
                                    nc.vector.tensor_scalar_add(
                                        shift[:], pos_sb[:, s:s + 1],
                                        float(-(qtr * 1024 + pn * 512)),
                                    )
                                    madd = pb.tile([128, 512], f32, tag="madd", bufs=1, name="madd")
                                    nc.vector.tensor_scalar(
                                        out=madd[:], in0=iota_f[:], scalar1=shift[:, 0:1],
                                        scalar2=NEG_BIG, op0=OP.is_gt, op1=OP.mult,
                                    )
                                    nc.vector.tensor_tensor(
                                        out=ssb[:, 512 * pn:512 * (pn + 1)], in0=ps[:],
                                        in1=madd[:], op=OP.add,
                                    )
                                else:
                                    nc.scalar.copy(ssb[:, 512 * pn:512 * (pn + 1)], ps[:])
                            if qtr == 0:
                                nc.vector.reduce_max(
                                    negm[s][:], ssb[:], axis=AX.X, negate=True)
                                nc.vector.tensor_scalar_add(negm[s][:], negm[s][:], -DELTA)
                            psb = pb.tile([128, 1024], bf16, tag="psb", bufs=3, name="psb")
                            nc.scalar.activation(
                                psb[:], ssb[:], AT.Exp, bias=negm[s][:, 0:1], scale=1.0,
                                accum_out=lpart[s][:, qtr:qtr + 1],
                            )
                            ps_tr = psp.tile([128, 1024], bf16, tag="ps_trrow", bufs=1, name="ps_trrow")
                            for j in range(8):
                                nc.tensor.transpose(
                                    ps_tr[:, 128 * j:128 * (j + 1)],
                                    psb[:, 128 * j:128 * (j + 1)], ident[:])
                            pt_sb = pb.tile([128, 1024], bf16, tag="pt_sb", bufs=9, name="pt_sb")
                            nc.scalar.copy(pt_sb[:], ps_tr[:])
                            pt_tiles[(s, qtr)] = pt_sb

                # ---- Pass 2: att @ v per quarter, then per-slot finalize ----
                for qtr in range(4):
                    with nc.named_scope(f"av{qtr}"):
                        vq = [pb.tile([128, 1024], bf16, tag=f"vq{j}", bufs=2, name=f"vq{j}") for j in range(8)]
                        for j in range(8):
                            eng = nc.sync if j % 2 == 0 else nc.gpsimd
                            eng.dma_start(
                                out=vq[j][:],
                                in_=vag_d[1024 * qtr + 128 * j:1024 * qtr + 128 * (j + 1), :],
                            )
                        for s in range(4):
                            if qtr >= PADQ[s]:
                                continue
                            last_q = (qtr == PADQ[s] - 1)
                            pt_sb = pt_tiles[(s, qtr)]
                            ps_av = psp.tile([128, 1024], f32, tag="ps_av", name="ps_av", bufs=2)
                            for h in range(2):
                                for j in range(8):
                                    nc.tensor.matmul(
                                        ps_av[:, 512 * h:512 * (h + 1)],
                                        pt_sb[:, 128 * j:128 * (j + 1)],
                                        vq[j][:, 512 * h:512 * (h + 1)],
                                        start=(j == 0), stop=(j == 7),
                                    )
                            if qtr == 0:
                                nc.vector.tensor_copy(A_sb[s][:], ps_av[:])
                            else:
                                nc.vector.tensor_tensor(
                                    out=A_sb[s][:], in0=A_sb[s][:], in1=ps_av[:], op=OP.add)

                            # ---------- finalize slot after its last quarter ----------
                            if last_q:
                                with nc.named_scope(f"fin{s}"):
                                    lsum = pb.tile([128, 1], f32, tag="lsum", bufs=2, name="lsum")
                                    if PADQ[s] > 1:
                                        nc.vector.reduce_sum(
                                            lsum[:], lpart[s][:, 0:PADQ[s]], axis=AX.X)
                                    else:
                                        nc.vector.tensor_copy(lsum[:], lpart[s][:, 0:1])
                                    rl = pb.tile([128, 1], f32, tag="rl", bufs=2, name="rl")
                                    nc.vector.reciprocal(rl[:], lsum[:])
                                    attn_f = pb.tile([128, D], f32, tag="attn_f", bufs=2, name="attn_f")
                                    nc.scalar.activation(
                                        attn_f[:], A_sb[s][:], AT.Copy, bias=0.0,
                                        scale=rl[:, 0:1])
                                    nc.sync.dma_start(
                                        out=attn_d[128 * s:128 * (s + 1), :], in_=attn_f[:])
                                    attn_b = pb.tile([128, D], bf16, tag="attn_b", bufs=2, name="attn_b")
                                    nc.vector.tensor_copy(attn_b[:], attn_f[:])
                                    ps_t2 = psp.tile([128, 1024], bf16, tag="ps_trrow", bufs=1, name="ps_trrow")
                                    for ec in range(8):
                                        nc.tensor.transpose(
                                            ps_t2[:, 128 * ec:128 * (ec + 1)],
                                            attn_b[:, 128 * ec:128 * (ec + 1)],
                                            ident[:])
                                    at_row = pb.tile([128, 1024], bf16, tag="at_sb", bufs=2, name="at_sb")
                                    nc.scalar.copy(at_row[:], ps_t2[:])
                                    xqr = pb.tile([128, D], f32, tag="xqr", bufs=2, name="xqr")
                                    nc.sync.dma_start(
                                        out=xqr[:], in_=xqres_d[128 * s:128 * (s + 1), :])
                                    out_sb = pb.tile([128, D], f32, tag="out_sb", bufs=2, name="out_sb")
                                    for h in range(2):
                                        ps_o = psp.tile([128, 512], f32, tag="pp", name="pp", bufs=3)
                                        for ec in range(8):
                                            nc.tensor.matmul(
                                                ps_o[:], at_row[:, 128 * ec:128 * (ec + 1)],
                                                wproj[ec][:, 512 * h:512 * (h + 1)],
                                                start=(ec == 0), stop=(ec == 7),
                                            )
                                        nc.vector.tensor_tensor(
                                            out=out_sb[:, 512 * h:512 * (h + 1)], in0=ps_o[:],
                                            in1=xqr[:, 512 * h:512 * (h + 1)], op=OP.add)
                                    nc.sync.dma_start(
                                        out=out_d[128 * s:128 * (s + 1), :], in_=out_sb[:])

    nc.compile()
    return nc


def _get_compiled():
    global _COMPILED
    if _COMPILED is None:
        _COMPILED = _build()
    return _COMPILED


def kernel(x, attention_mask, Wq, Wkv, Wproj, _trace=False):
    global LAST_EXEC_NS
    from concourse.bass_utils import run_bass_kernel_spmd

    x = np.asarray(x)
    attention_mask = np.asarray(attention_mask)
    Wq, Wkv, Wproj = np.asarray(Wq), np.asarray(Wkv), np.asarray(Wproj)
    assert x.shape == (T, D) and attention_mask.shape == (T,)
    assert np.array_equal(attention_mask, np.arange(T, dtype=attention_mask.dtype)), \
        "kernel assumes attention_mask == arange(T)"

    x16 = x.astype(np.float16)
    wqT = np.ascontiguousarray(Wq.T).astype(np.float16)
    wkT = np.ascontiguousarray(Wkv[:D].T).astype(np.float16)
    wvT = np.ascontiguousarray(Wkv[D:].T).astype(np.float16)
    wpT = np.ascontiguousarray(Wproj.T).astype(ml_dtypes.bfloat16)

    in_maps = []
    core_rows = []
    for c in range(N_CORES):
        blocks = core_blocks(c)
        rows = np.concatenate([np.arange(128 * b, 128 * (b + 1)) for b in blocks])
        core_rows.append(rows)
        xq = x[rows]                      # [512, D] f32
        xqT = np.ascontiguousarray(x16[rows].T)    # [D, 512] f16
        xkvT = np.ascontiguousarray(x16[512 * c:512 * (c + 1)].T)
        pos = np.empty((128, 4), np.float32)
        for s, b in enumerate(blocks):
            pos[:, s] = 128 * b + np.arange(128)
        in_maps.append({
            "xqT": xqT, "xkvT": xkvT,
            "xqres": np.ascontiguousarray(xq.astype(np.float32)),
            "pos": pos,
            "wqT": wqT, "wkT": wkT, "wvT": wvT, "wpT": wpT,
        })

    nc = _get_compiled()
    res = run_bass_kernel_spmd(nc, in_maps, list(range(N_CORES)), trace=_trace)
    LAST_EXEC_NS = res.exec_time_ns
    globals()["LAST_RES"] = res

    out_full = np.empty((T, D), np.float32)
    x_new = x.astype(np.float32).copy()
    for c in range(N_CORES):
        r = res.results[c]
        out_full[core_rows[c]] = r["out"]
        x_new[core_rows[c]] += r["attn"]
    return out_full, x_new



# revision 19
# speedup vs baseline: 1.1812x; 1.0199x over previous
"""Trainium2 Bass kernel for nn_CharAttention (causal single-head attention, T=4096, D=1024).

Strategy (8 NeuronCores, SPMD):
  - Queries sharded across cores with a balanced causal interleave: core c owns
    global 128-row q-blocks {c, 15-c, 16+c, 31-c} ("slots" 0..3), so every core
    does the same causal work (structurally identical static program).
  - k/v computed shard-wise (core c projects rows [512c, 512c+512)) then
    AllGather'd in fp16/bf16.
  - Slot s is padded to (s+1)*1024 key-columns; the data-dependent causal
    boundary is applied with an iota>pos additive -1e9 mask on the last quarter
    of each slot (the diagonal always lands there for every core).
  - Softmax without a running max: m_hat = rowmax(first 1024 cols) + 50.
    exp(s - m_hat) stays within bf16/f32 range (margin analysis: overflow needs
    a later-quarter score 138 above the quarter-0 max; underfow drops only
    weights < 1e-16 of the total), so quarter contributions accumulate with
    plain adds and one final 1/l normalization.
  - dtypes: q/k/scores chain in fp16 (PE full rate, 8x finer mantissa than
    bf16 -- scores have std ~32 and softmax is argmax-sensitive); p/v/attn/proj
    in bf16 (needs fp32-wide exponent range for the shifted exp).
"""

import numpy as np
import ml_dtypes

T = 4096
D = 1024
N_CORES = 8
NBLK = T // 128  # 32 global q-blocks
DELTA = 50.0
NEG_BIG = -1e9

# slot assignment: core c -> global blocks [c, 15-c, 16+c, 31-c]
def core_blocks(c):
    return [c, 15 - c, 16 + c, 31 - c]

PADQ = [1, 2, 3, 4]  # quarters (1024 cols) computed per slot

FILL_AGK = 0  # filler matmuls (~213ns each) bridging the AG(k) wait
FILL_AGV = 0  # filler matmuls bridging the AG(v) wait

_COMPILED = None
LAST_EXEC_NS = None


def _build():
    import concourse.bass as bass
    import concourse.mybir as mybir
    from concourse import bacc
    from concourse.tile import TileContext
    from concourse.masks import make_identity

    f16, bf16, f32 = mybir.dt.float16, mybir.dt.bfloat16, mybir.dt.float32
    AT = mybir.ActivationFunctionType
    OP = mybir.AluOpType
    AX = mybir.AxisListType

    nc = bacc.Bacc("TRN2", target_bir_lowering=False, debug=False, num_devices=N_CORES)

    # --- I/O ---
    xqT_d = nc.dram_tensor("xqT", [D, 512], f16, kind="ExternalInput")
    xkvT_d = nc.dram_tensor("xkvT", [D, 512], f16, kind="ExternalInput")
    xqres_d = nc.dram_tensor("xqres", [512, D], f32, kind="ExternalInput")
    pos_d = nc.dram_tensor("pos", [128, 4], f32, kind="ExternalInput")
    wqT_d = nc.dram_tensor("wqT", [D, D], f16, kind="ExternalInput")
    wkT_d = nc.dram_tensor("wkT", [D, D], f16, kind="ExternalInput")
    wvT_d = nc.dram_tensor("wvT", [D, D], f16, kind="ExternalInput")
    wpT_d = nc.dram_tensor("wpT", [D, D], bf16, kind="ExternalInput")
    out_d = nc.dram_tensor("out", [512, D], f32, kind="ExternalOutput")
    attn_d = nc.dram_tensor("attn", [512, D], f32, kind="ExternalOutput")

    # internal DRAM for the collective
    ktloc_d = nc.dram_tensor("ktloc", [D, 512], f16)
    vloc_d = nc.dram_tensor("vloc", [512, D], bf16)
    ktag_d = nc.dram_tensor("ktag", [N_CORES, D, 512], f16, addr_space="Shared")
    vag_d = nc.dram_tensor("vag", [T, D], bf16, addr_space="Shared")
    dumloc_d = nc.dram_tensor("dumloc", [128, 4], f32)
    dumag_d = nc.dram_tensor("dumag", [N_CORES, 128, 4], f32, addr_space="Shared")

    groups = [list(range(N_CORES))]

    with TileContext(nc) as tc:
        with (
            tc.tile_pool(name="persist", bufs=1) as pp,
            tc.tile_pool(name="psum", bufs=1, space="PSUM") as psp,
        ):
            # persistent tiles
            warm_sb = pp.tile([128, 512], f16, tag="warm_sb", name="warm_sb")
            nc.vector.memset(warm_sb[:], 0.0)
            # HAM warmup: real matmuls (transposes don't count as PE-busy for
            # HAM) with no DMA deps, so the PE reaches K=8/8 before k_proj and
            # stays busy during the input-DMA lead-in.
            for w in range(6):
                wps = psp.tile([128, 512], f32, tag="pp", name="pp", bufs=3)
                nc.tensor.matmul(wps[:], warm_sb[:, 0:128], warm_sb[:], start=True, stop=True)

            qt = [pp.tile([128, 512], f16, tag=f"qt{e}", name=f"qt{e}") for e in range(8)]
            ident = pp.tile([128, 128], bf16, tag="ident", name="ident")
            make_identity(nc, ident[:])
            iota_i = pp.tile([128, 512], mybir.dt.int32, tag="iota_i", name="iota_i")
            nc.gpsimd.iota(iota_i[:], pattern=[[1, 512]], base=0, channel_multiplier=0)
            iota_f = pp.tile([128, 512], f32, tag="iota_f", name="iota_f")
            nc.vector.tensor_copy(iota_f[:], iota_i[:])
            pos_sb = pp.tile([128, 4], f32, tag="pos_sb", name="pos_sb")
            nc.sync.dma_start(out=pos_sb[:], in_=pos_d[:])
            A_sb = [pp.tile([128, D], f32, tag=f"A{s}", name=f"A{s}") for s in range(4)]
            lpart = [pp.tile([128, 4], f32, tag=f"lp{s}", name=f"lp{s}") for s in range(4)]
            negm = [pp.tile([128, 1], f32, tag=f"nm{s}", name=f"nm{s}") for s in range(4)]

            # ---------------- Phase A: projections + allgather ----------------
            with tc.tile_pool(name="pha", bufs=1) as pa:
                # per-chunk loads, dispatch spread across engine queues
                # (sync: xkvt, scalar: wk, vector: wv, gpsimd: xq/wq)
                xkvt = [pa.tile([128, 512], f16, tag=f"xkvt{d}", name=f"xkvt{d}") for d in range(8)]
                wk = [pa.tile([128, D], f16, tag=f"wk{d}", name=f"wk{d}") for d in range(8)]
                wv = [pa.tile([128, D], f16, tag=f"wv{d}", name=f"wv{d}") for d in range(8)]
                xqt = [pa.tile([128, 512], f16, tag=f"xqt{d}", name=f"xqt{d}") for d in range(8)]
                wq = [pa.tile([128, D], f16, tag=f"wq{d}", name=f"wq{d}") for d in range(8)]
                for d in range(8):
                    nc.sync.dma_start(out=xkvt[d][:], in_=xkvT_d[128 * d:128 * (d + 1), :])
                    nc.scalar.dma_start(out=wk[d][:], in_=wkT_d[128 * d:128 * (d + 1), :])
                for d in range(8):
                    nc.gpsimd.dma_start(out=wv[d][:], in_=wvT_d[128 * d:128 * (d + 1), :])
                for d in range(8):
                    nc.gpsimd.dma_start(out=xqt[d][:], in_=xqT_d[128 * d:128 * (d + 1), :])
                    nc.gpsimd.dma_start(out=wq[d][:], in_=wqT_d[128 * d:128 * (d + 1), :])

                with nc.named_scope("k_proj"):
                    # kT_local[e,t] = sum_d WkT[d,e] * xkvT[d,t]
                    for e in range(8):
                        ps = psp.tile([128, 512], f32, tag="pp", name="pp", bufs=3)
                        for d in range(8):
                            nc.tensor.matmul(
                                ps[:], wk[d][:, 128 * e:128 * (e + 1)], xkvt[d][:],
                                start=(d == 0), stop=(d == 7),
                            )
                        kt_sb = pa.tile([128, 512], f16, tag="kt_sb", name="kt_sb", bufs=3)
                        nc.scalar.copy(kt_sb[:], ps[:])
                        nc.sync.dma_start(out=ktloc_d[128 * e:128 * (e + 1), :], in_=kt_sb[:])

                nc.gpsimd.collective_compute(
                    "AllGather", mybir.AluOpType.bypass, replica_groups=groups,
                    ins=[ktloc_d[:]], outs=[ktag_d[:]],
                )

                with nc.named_scope("v_proj"):
                    # v_local[t,e] = sum_d xkvT[d,t] * WvT[d,e]
                    for t in range(4):
                        for h in range(2):
                            ps = psp.tile([128, 512], f32, tag="pp", name="pp", bufs=3)
                            for d in range(8):
                                nc.tensor.matmul(
                                    ps[:], xkvt[d][:, 128 * t:128 * (t + 1)],
                                    wv[d][:, 512 * h:512 * (h + 1)],
                                    start=(d == 0), stop=(d == 7),
                                )
                            v_sb = pa.tile([128, 512], bf16, tag="v_sb", name="v_sb", bufs=3)
                            nc.scalar.copy(v_sb[:], ps[:])
                            nc.sync.dma_start(
                                out=vloc_d[128 * t:128 * (t + 1), 512 * h:512 * (h + 1)],
                                in_=v_sb[:],
                            )

                nc.gpsimd.collective_compute(
                    "AllGather", mybir.AluOpType.bypass, replica_groups=groups,
                    ins=[vloc_d[:]], outs=[vag_d[:]],
                )

                with nc.named_scope("q_proj"):
                    for e in range(8):
                        ps = psp.tile([128, 512], f32, tag="pp", name="pp", bufs=3)
                        for d in range(8):
                            nc.tensor.matmul(
                                ps[:], wq[d][:, 128 * e:128 * (e + 1)], xqt[d][:],
                                start=(d == 0), stop=(d == 7),
                            )
                        nc.scalar.copy(qt[e][:], ps[:])

            # ---------------- Phase B: attention over quarters ----------------
            with tc.tile_pool(name="phb", bufs=1) as pb:
                wproj = [pb.tile([128, D], bf16, tag=f"wp{d}", name=f"wp{d}") for d in range(8)]
                for d in range(8):
                    nc.sync.dma_start(out=wproj[d][:], in_=wpT_d[128 * d:128 * (d + 1), :])

                # ---- Pass 1: all scores + exp + P-transposes (overlaps v-AllGather) ----
                pt_tiles = {}
                for qtr in range(4):
                    with nc.named_scope(f"sc{qtr}"):
                        kq = [pb.tile([128, 1024], f16, tag=f"kq{e}", bufs=4, name=f"kq{e}") for e in range(8)]
                        for e in range(8):
                            eng = nc.sync if e % 2 == 0 else nc.gpsimd
                            for hh in range(2):
                                eng.dma_start(
                                    out=kq[e][:, 512 * hh:512 * (hh + 1)],
                                    in_=ktag_d[2 * qtr + hh, 128 * e:128 * (e + 1), :],
                                )
                        for s in range(4):
                            if qtr >= PADQ[s]:
                                continue
                            last_q = (qtr == PADQ[s] - 1)
                            ssb = pb.tile([128, 1024], f32, tag="ssb", bufs=2, name="ssb")
                            for pn in range(2):
                                ps = psp.tile([128, 512], f32, tag="pp", name="pp", bufs=3)
                                for e in range(8):
                                    nc.tensor.matmul(
                                        ps[:], qt[e][:, 128 * s:128 * (s + 1)],
                                        kq[e][:, 512 * pn:512 * (pn + 1)],
                                        start=(e == 0), stop=(e == 7),
                                    )
                                if last_q:
                                    shift = pb.tile([128, 1], f32, tag="shift", bufs=2, name="shift")
                                    nc.vector.tensor_scalar_add(
                                        shift[:], pos_sb[:, s:s + 1],
                                        float(-(qtr * 1024 + pn * 512)),
                                    )
                                    madd = pb.tile([128, 512], f32, tag="madd", bufs=1, name="madd")
                                    nc.vector.tensor_scalar(
                                        out=madd[:], in0=iota_f[:], scalar1=shift[:, 0:1],
                                        scalar2=NEG_BIG, op0=OP.is_gt, op1=OP.mult,
                                    )
                                    nc.vector.tensor_tensor(
                                        out=ssb[:, 512 * pn:512 * (pn + 1)], in0=ps[:],
                                        in1=madd[:], op=OP.add,
                                    )
                                else:
                                    nc.scalar.copy(ssb[:, 512 * pn:512 * (pn + 1)], ps[:])
                            if qtr == 0:
                                nc.vector.reduce_max(
                                    negm[s][:], ssb[:], axis=AX.X, negate=True)
                                nc.vector.tensor_scalar_add(negm[s][:], negm[s][:], -DELTA)
                            psb = pb.tile([128, 1024], bf16, tag="psb", bufs=3, name="psb")
                            nc.scalar.activation(
                                psb[:], ssb[:], AT.Exp, bias=negm[s][:, 0:1], scale=1.0,
                                accum_out=lpart[s][:, qtr:qtr + 1],
                            )
                            ps_tr = psp.tile([128, 1024], bf16, tag="ps_trrow", bufs=1, name="ps_trrow")
                            for j in range(8):
                                nc.tensor.transpose(
                                    ps_tr[:, 128 * j:128 * (j + 1)],
                                    psb[:, 128 * j:128 * (j + 1)], ident[:])
                            pt_sb = pb.tile([128, 1024], bf16, tag="pt_sb", bufs=9, name="pt_sb")
                            nc.scalar.copy(pt_sb[:], ps_tr[:])
                            pt_tiles[(s, qtr)] = pt_sb

                # ---- Pass 2: att @ v per quarter, then per-slot finalize ----
                for qtr in range(4):
                    with nc.named_scope(f"av{qtr}"):
                        vq = [pb.tile([128, 1024], bf16, tag=f"vq{j}", bufs=2, name=f"vq{j}") for j in range(8)]
                        for j in range(8):
                            eng = nc.sync if j % 2 == 0 else nc.gpsimd
                            eng.dma_start(
                                out=vq[j][:],
                                in_=vag_d[1024 * qtr + 128 * j:1024 * qtr + 128 * (j + 1), :],
                            )
                        for s in range(4):
                            if qtr >= PADQ[s]:
                                continue
                            last_q = (qtr == PADQ[s] - 1)
                            pt_sb = pt_tiles[(s, qtr)]
                            ps_av = psp.tile([128, 1024], f32, tag="ps_av", name="ps_av", bufs=2)
                            for h in range(2):
                                for j in range(8):
                                    nc.tensor.matmul(
                                        ps_av[:, 512 * h:512 * (h + 1)],
                                        pt_sb[:, 128 * j:128 * (j + 1)],
                                        vq[j][:, 512 * h:512 * (h + 1)],
                                        start=(j == 0), stop=(j == 7),
                                    )
                            if qtr == 0:
                                nc.vector.tensor_copy(A_sb[s][:], ps_av[:])
                            else:
                                nc.vector.tensor_tensor(
                                    out=A_sb[s][:], in0=A_sb[s][:], in1=ps_av[:], op=OP.add)

                            # ---------- finalize slot after its last quarter ----------
                            if last_q:
                                with nc.named_scope(f"fin{s}"):
                                    lsum = pb.tile([128, 1], f32, tag="lsum", bufs=2, name="lsum")
                                    if PADQ[s] > 1:
                                        nc.vector.reduce_sum(
                                            lsum[:], lpart[s][:, 0:PADQ[s]], axis=AX.X)
                                    else:
                                        nc.vector.tensor_copy(lsum[:], lpart[s][:, 0:1])
                                    rl = pb.tile([128, 1], f32, tag="rl", bufs=2, name="rl")
                                    nc.vector.reciprocal(rl[:], lsum[:])
                                    attn_f = pb.tile([128, D], f32, tag="attn_f", bufs=2, name="attn_f")
                                    nc.scalar.activation(
                                        attn_f[:], A_sb[s][:], AT.Copy, bias=0.0,
                                        scale=rl[:, 0:1])
                                    nc.sync.dma_start(
                                        out=attn_d[128 * s:128 * (s + 1), :], in_=attn_f[:])
                                    attn_b = pb.tile([128, D], bf16, tag="attn_b", bufs=2, name="attn_b")
                                    nc.vector.tensor_copy(attn_b[:], attn_f[:])
                                    ps_t2 = psp.tile([128, 1024], bf16, tag="ps_trrow", bufs=1, name="ps_trrow")
                                    for ec in range(8):
                                        nc.tensor.transpose(
                                            ps_t2[:, 128 * ec:128 * (ec + 1)],
                                            attn_b[:, 128 * ec:128 * (ec + 1)],
                                            ident[:])
                                    at_row = pb.tile([128, 1024], bf16, tag="at_sb", bufs=2, name="at_sb")
                                    nc.scalar.copy(at_row[:], ps_t2[:])
                                    xqr = pb.tile([128, D], f32, tag="xqr", bufs=2, name="xqr")
                                    nc.sync.dma_start(
                                        out=xqr[:], in_=xqres_d[128 * s:128 * (s + 1), :])
                                    out_sb = pb.tile([128, D], f32, tag="out_sb", bufs=2, name="out_sb")
                                    for h in range(2):
                                        ps_o = psp.tile([128, 512], f32, tag="pp", name="pp", bufs=3)
                                        for ec in range(8):
                                            nc.tensor.matmul(
                                                ps_o[:], at_row[:, 128 * ec:128 * (ec + 1)],
                                                wproj[ec][:, 512 * h:512 * (h + 1)],
                                                start=(ec == 0), stop=(ec == 7),
                                            )
                                        nc.vector.tensor_tensor(
                                            out=out_sb[:, 512 * h:512 * (h + 1)], in0=ps_o[:],
                                            in1=xqr[:, 512 * h:512 * (h + 1)], op=OP.add)
                                    nc.sync.dma_start(
                                        out=out_d[128 * s:128 * (s + 1), :], in_=out_sb[:])

    nc.compile()
    return nc


def _get_compiled():
    global _COMPILED
    if _COMPILED is None:
        _COMPILED = _build()
    return _COMPILED


def kernel(x, attention_mask, Wq, Wkv, Wproj, _trace=False):
    global LAST_EXEC_NS
    from concourse.bass_utils import run_bass_kernel_spmd

    x = np.asarray(x)
    attention_mask = np.asarray(attention_mask)
    Wq, Wkv, Wproj = np.asarray(Wq), np.asarray(Wkv), np.asarray(Wproj)
    assert x.shape == (T, D) and attention_mask.shape == (T,)
    assert np.array_equal(attention_mask, np.arange(T, dtype=attention_mask.dtype)), \
        "kernel assumes attention_mask == arange(T)"

    x16 = x.astype(np.float16)
    wqT = np.ascontiguousarray(Wq.T).astype(np.float16)
    wkT = np.ascontiguousarray(Wkv[:D].T).astype(np.float16)
    wvT = np.ascontiguousarray(Wkv[D:].T).astype(np.float16)
    wpT = np.ascontiguousarray(Wproj.T).astype(ml_dtypes.bfloat16)

    in_maps = []
    core_rows = []
    for c in range(N_CORES):
        blocks = core_blocks(c)
        rows = np.concatenate([np.arange(128 * b, 128 * (b + 1)) for b in blocks])
        core_rows.append(rows)
        xq = x[rows]                      # [512, D] f32
        xqT = np.ascontiguousarray(x16[rows].T)    # [D, 512] f16
        xkvT = np.ascontiguousarray(x16[512 * c:512 * (c + 1)].T)
        pos = np.empty((128, 4), np.float32)
        for s, b in enumerate(blocks):
            pos[:, s] = 128 * b + np.arange(128)
        in_maps.append({
            "xqT": xqT, "xkvT": xkvT,
            "xqres": np.ascontiguousarray(xq.astype(np.float32)),
            "pos": pos,
            "wqT": wqT, "wkT": wkT, "wvT": wvT, "wpT": wpT,
        })

    nc = _get_compiled()
    res = run_bass_kernel_spmd(nc, in_maps, list(range(N_CORES)), trace=_trace)
    LAST_EXEC_NS = res.exec_time_ns
    globals()["LAST_RES"] = res

    out_full = np.empty((T, D), np.float32)
    x_new = x.astype(np.float32).copy()
    for c in range(N_CORES):
        r = res.results[c]
        out_full[core_rows[c]] = r["out"]
        x_new[core_rows[c]] += r["attn"]
    return out_full, x_new

